# revision 3
# baseline (speedup 1.0000x reference)
"""Trainium2 Bass kernel for the MACE-style SymmetricContraction MessageBlock.

Math (per sample s=(b,c), x = a_i[b,c,:] in R^16, w*[b,k,c] = node_attrs @ W*):
  out0[b,c]   = sum U3_l0[i1,i2,i3,k] x_i1 x_i2 x_i3 w3[k]
              + sum U2_l0[i1,i2,k]    x_i1 x_i2      w2[k]
              + sum U1_l0[i1,k]       x_i1           w1[k]
  out1[b,c,l] = same with the l1 U/W tensors.

Device pipeline per 512-sample chunk. Pair monomials x_a*x_b come from the
sum-of-squares identity x_a x_b = ((x_a+x_b)^2 - x_a^2 - x_b^2)/2; the x_a^2
corrections are folded into the coefficient rows on the host:
  sq   = square(SelAB.T @ x)                 # 120 rows (PE+ACT)
  xin  = [x ; 1 ; x^2]                       # 33 rows; x^2 by ACT per chunk
  S1   = M1a'.T @ sq + M1bc'.T @ xin         # 431 cols, 4 m-chunks of <=119
  Z    = S1 * Xrep (bf16)                    # Xrep = x_{i1(col)} via bf16
                                             #   broadcast SBUF->SBUF DMA
  P41  = G1.T @ Z                            # reduce over i1 -> 41 cols
  ZW   = P41 * wt41                          # wt41 pre-expanded at setup
  out4 = G2.T @ ZW                           # out0, out1 l=0..2

Engine split per chunk: PE 14 matmul passes; ACT sq square + out copy;
Pool x^2 multiply; DVE 4 Z mults + ZW; DMA streams x, broadcasts Xrep (f32r).

Sharding: data-parallel over nodes, 128 nodes per core on 8 cores.
"""
import numpy as np

B, C, DIM_I, E = 1024, 128, 16, 10
NCORES = 8
BPC = B // NCORES
S_PER_CORE = BPC * C
CHUNK = 512

OFFDIAG = [(a, b) for a in range(DIM_I) for b in range(a + 1, DIM_I)]  # 120
NOFF = len(OFFDIAG)
NC_CUBIC = 26
NCOLS_C = DIM_I * NC_CUBIC          # 416
NCOLS_S1 = NCOLS_C + 11 + 4         # 431
NP41 = 41
NW = 19
# m-chunks of 4 i1-groups (4*26=104); last chunk also carries quad/lin cols
MCHUNKS = [(0, 104), (104, 208), (208, 312), (312, NCOLS_S1)]
NXIN = 49  # xin rows: 0..15 x^2, 16..31 zero, 32..47 x, 48 ones

# P41 col -> packed w row (w3_l0:0-4, w2_l0:5-6, w1_l0:7, w3_l1:8-14,
# w2_l1:15-17, w1_l1:18)
KROW = [0] * NP41
for _m in range(5):
    KROW[_m] = _m
for _l in range(3):
    for _k in range(7):
        KROW[5 + _l * 7 + _k] = 8 + _k
KROW[26], KROW[27] = 5, 6
for _l in range(3):
    for _k in range(3):
        KROW[28 + _l * 3 + _k] = 15 + _k
KROW[37] = 7
for _l in range(3):
    KROW[38 + _l] = 18


# ---------------------------------------------------------------- host consts
def _pair_coeff(U3_l0, U3_l1, U2_l0, U2_l1, a, b):
    """431-col coefficient row for the pair monomial x_a*x_b (a<=b)."""
    dup = a < b
    row = np.zeros(NCOLS_S1, np.float32)
    for i1 in range(DIM_I):
        c0 = i1 * NC_CUBIC
        row[c0:c0 + 5] = U3_l0[i1, a, b] + (U3_l0[i1, b, a] if dup else 0)
        for l in range(3):
            row[c0 + 5 + l * 7: c0 + 12 + l * 7] = (
                U3_l1[l, i1, a, b] + (U3_l1[l, i1, b, a] if dup else 0))
    row[416:418] = U2_l0[a, b] + (U2_l0[b, a] if dup else 0)
    for l in range(3):
        row[418 + l * 3: 421 + l * 3] = (
            U2_l1[l, a, b] + (U2_l1[l, b, a] if dup else 0))
    return row


def _build_consts(U3_l0, U2_l0, U1_l0, U3_l1, U2_l1, U1_l1):
    M1a_raw = np.zeros((NOFF, NCOLS_S1), np.float32)
    for p, (a, b) in enumerate(OFFDIAG):
        M1a_raw[p] = _pair_coeff(U3_l0, U3_l1, U2_l0, U2_l1, a, b)
    # sum-of-squares substitution: sq rows carry M1a/2; diagonal rows get
    # the -1/2 sum of all off-diag rows touching that index
    M1a = 0.5 * M1a_raw
    M1bc = np.zeros((NXIN, NCOLS_S1), np.float32)
    for i in range(DIM_I):
        corr = np.zeros(NCOLS_S1, np.float32)
        for p, (a, b) in enumerate(OFFDIAG):
            if a == i or b == i:
                corr += M1a_raw[p]
        # row i: x_i^2 coefficient
        M1bc[i] = _pair_coeff(U3_l0, U3_l1, U2_l0, U2_l1, i, i) - 0.5 * corr
    for i in range(DIM_I):
        # rows 32..47: linear x_i coefficients
        M1bc[32 + i, 427] = U1_l0[i, 0]
        for l in range(3):
            M1bc[32 + i, 428 + l] = U1_l1[l, i, 0]

    G1 = np.zeros((NCOLS_S1, NP41), np.float32)
    for i1 in range(DIM_I):
        for m in range(NC_CUBIC):
            G1[i1 * NC_CUBIC + m, m] = 1
    for j in range(11):
        G1[416 + j, 26 + j] = 1
    for j in range(4):
        G1[427 + j, 37 + j] = 1

    G2 = np.zeros((NP41, 4), np.float32)
    G2[0:5, 0] = 1
    G2[26:28, 0] = 1
    G2[37, 0] = 1
    for l in range(3):
        G2[5 + l * 7: 12 + l * 7, 1 + l] = 1
        G2[28 + l * 3: 31 + l * 3, 1 + l] = 1
        G2[38 + l, 1 + l] = 1

    SelAB = np.zeros((17, NOFF), np.float32)
    for p, (a, b) in enumerate(OFFDIAG):
        SelAB[a, p] += 1
        SelAB[b, p] += 1
    return dict(M1a=M1a, M1bc=M1bc, G1=G1, G2=G2, SelAB=SelAB)


# ---------------------------------------------------------------- bass program
def build_nc(bpc=BPC, repeat=1):
    import concourse.bass as bass
    import concourse.bacc as bacc
    import concourse.mybir as mybir
    import concourse.tile as tile

    s_core = bpc * C
    nchunk = s_core // CHUNK
    f32 = mybir.dt.float32
    f32r = mybir.dt.float32r
    bf16 = mybir.dt.bfloat16
    MUL = mybir.AluOpType.mult

    nc = bacc.Bacc("TRN2", target_bir_lowering=False, debug=False)

    xa_d = nc.dram_tensor("xa", [17, s_core], f32r, kind="ExternalInput")
    wt41_d = nc.dram_tensor("Wt41", [NP41, s_core], bf16, kind="ExternalInput")
    # packed f32r consts: m1a | m1bc | sab | g2 along the free dim
    cf_d = nc.dram_tensor("CF", [NOFF, 2 * NCOLS_S1 + NOFF + 4 + 4 * NP41],
                          f32r, kind="ExternalInput")
    zf_d = nc.dram_tensor("ZfillC", [16, CHUNK * 4], f32r, kind="ExternalInput")
    out_d = nc.dram_tensor("out", [bpc, 4, C], f32, kind="ExternalOutput")

    SUP = 4
    SCH = CHUNK * SUP

    with tile.TileContext(nc) as tc:
        with (
            tc.tile_pool(name="const", bufs=1) as cp,
            tc.tile_pool(name="xbfp", bufs=3) as xbfp,
            tc.tile_pool(name="sqp", bufs=4) as sqp,
            tc.tile_pool(name="s1cp", bufs=3) as s1cp,
            tc.tile_pool(name="xrp", bufs=2) as xrp,
            tc.tile_pool(name="zp", bufs=8) as zp,
            tc.tile_pool(name="zwp", bufs=3) as zwp,
            tc.tile_pool(name="obp", bufs=3) as obp,
            tc.tile_pool(name="pab", bufs=1, space="PSUM") as pab,
            tc.tile_pool(name="ps1", bufs=3, space="PSUM") as ps1,
            tc.tile_pool(name="pp41", bufs=3, space="PSUM") as pp41,
            tc.tile_pool(name="po", bufs=1, space="PSUM") as po,
        ):
            def ctile(name, shape, dram, dt=f32r):
                t = cp.tile(shape, dt, tag=name)
                nc.sync.dma_start(t[:, :], dram[:])
                return t

            cf = ctile("cf", [NOFF, 2 * NCOLS_S1 + NOFF + 4 + 4 * NP41], cf_d)
            m1a = cf[:, 0:NCOLS_S1]
            m1bc = cf[0:NXIN, NCOLS_S1:2 * NCOLS_S1]

            g2 = cf[0:NP41, 2 * NCOLS_S1 + NOFF:2 * NCOLS_S1 + NOFF + 4]
            sab = cf[0:17, 2 * NCOLS_S1:2 * NCOLS_S1 + NOFF]
            CG0 = 2 * NCOLS_S1 + NOFF + 4
            g1t = [cf[0:MCHUNKS[t][1] - MCHUNKS[t][0],
                      CG0 + NP41 * t:CG0 + NP41 * (t + 1)] for t in range(4)]

            # per-node path weights, precomputed on the host. DMA issued
            # after the first superchunk's input streams (ZW needs it last).
            wt41 = cp.tile([NP41, s_core], bf16, tag="wt41")

            # xin ring: [x^2 ; 0 ; x ; 1] per superchunk. Manual ring so the
            # zero band (read by M1bc rows 16:32, which are all-zero coeffs)
            # is written once; engine writes stay quadrant-aligned.
            xin_ring = []
            for i in range(3):
                xt = cp.tile([NXIN, SCH], f32r, tag=f"xin{i}")
                nc.scalar.dma_start(xt[16:32, :], zf_d[:])
                xin_ring.append(xt)

            def issue_inputs(sc):
                """Stream xin/xbf + Xrep broadcasts for superchunk sc."""
                ssl = slice(SCH * sc, SCH * (sc + 1))
                xin = xin_ring[issue_inputs.n % 3]
                issue_inputs.n += 1
                nc.sync.dma_start(xin[32:49, :], xa_d[:, ssl])
                xbf = xbfp.tile([17, SCH], f32r, tag="xbf")
                nc.scalar.dma_start(xbf[:, :], xa_d[:, ssl])
                xr = xrp.tile([119, 4 * SCH], f32r, tag="xr")
                for t_i in range(4):
                    base = xbf[4 * t_i:4 * t_i + 4, :]
                    srcap = bass.AP(tensor=base.tensor, offset=base.offset,
                                    ap=[list(base.ap[0]), [0, NC_CUBIC],
                                        list(base.ap[-1])])
                    eng = nc.sync if t_i % 2 == 0 else nc.scalar
                    eng.dma_start(xr[0:104, SCH * t_i:SCH * (t_i + 1)], srcap)
                ones = xbf[16:17, :]
                onesap = bass.AP(tensor=ones.tensor, offset=ones.offset,
                                 ap=[list(ones.ap[0]), [0, 15],
                                     list(ones.ap[-1])])
                nc.scalar.dma_start(xr[104:119, 3 * SCH:4 * SCH], onesap)
                xrt = [xr[0:MCHUNKS[t][1] - MCHUNKS[t][0],
                          SCH * t:SCH * (t + 1)] for t in range(4)]
                return xin, xbf, xrt

            def issue_head(xin_t, xbf_t, csl_t):
                """Chunk head: pair sums (PE), x^2 rows and sq (ACT). x^2 is
                squared from the bf16 x copy: it lives on partitions 0:16,
                matching the destination (engines cannot shift partitions).
                Issued one chunk ahead of the S1 stage that uses it."""
                psAB = pab.tile([NOFF, CHUNK], f32, tag="psAB")
                nc.tensor.matmul(psAB[:, :], sab[:, :], xbf_t[0:17, csl_t],
                                 start=True, stop=True)
                nc.gpsimd.tensor_tensor(xin_t[0:16, csl_t],
                                        xbf_t[0:16, csl_t],
                                        xbf_t[0:16, csl_t], MUL)
                sq = sqp.tile([NOFF, CHUNK], f32r, tag="sq")
                nc.scalar.square(sq[:, :], psAB[:, :])
                return sq

            # ---- main loop over superchunks of SUP chunks. Input streams
            # are issued one superchunk ahead (so they never queue behind
            # output DMAs) and chunk heads one chunk ahead (so sq/x^2 are
            # ready when S1 runs).
            scs = [s for _ in range(repeat) for s in range(nchunk // SUP)]
            issue_inputs.n = 0
            pending = issue_inputs(scs[0])
            nc.sync.dma_start(wt41[:, :], wt41_d[:])
            sq_next = issue_head(pending[0], pending[1], slice(0, CHUNK))
            pending_out = None
            for si, sc in enumerate(scs):
                xin, xbf, xrt = pending
                if si + 1 < len(scs):
                    pending = issue_inputs(scs[si + 1])
                outB = obp.tile([4, SCH], f32, tag="outB")

                for cc in range(SUP):
                    ch = SUP * sc + cc
                    sl = slice(CHUNK * ch, CHUNK * (ch + 1))
                    csl = slice(CHUNK * cc, CHUNK * (cc + 1))
                    sq = sq_next

                    # previous chunk's output copy + (at superchunk ends) its
                    # output DMA — pipelined here so the ACT queue never
                    # blocks on a not-yet-finished psO
                    if pending_out is not None:
                        p_psO, p_outB, p_csl, p_flush = pending_out
                        nc.scalar.copy(p_outB[:, p_csl], p_psO[:, :])
                        if p_flush is not None:
                            p_b0, p_nb = p_flush
                            nc.sync.dma_start(
                                out_d[p_b0:p_b0 + p_nb].rearrange(
                                    "b f c -> f b c"),
                                p_outB[:, :])

                    zt = []
                    for t_i, (c0, c1) in enumerate(MCHUNKS):
                        m = c1 - c0
                        psS = ps1.tile([m, CHUNK], f32, tag="s1")
                        nc.tensor.matmul(psS[:, :], m1a[:, c0:c1], sq[:, :],
                                         start=True, stop=False)
                        nc.tensor.matmul(psS[:, :], m1bc[:, c0:c1],
                                         xin[:, csl], start=False, stop=True)
                        z = zp.tile([m, CHUNK], f32r, tag="z")
                        nc.vector.scalar_tensor_tensor(
                            z[:, :], psS[:, :], 1.0, xrt[t_i][:, csl],
                            MUL, MUL)
                        zt.append(z)
                        if t_i == 1:
                            # head of the next chunk, early enough that its
                            # ACT squares land before that chunk's S1
                            if cc + 1 < SUP:
                                sq_next = issue_head(
                                    xin, xbf, slice(CHUNK * (cc + 1),
                                                    CHUNK * (cc + 2)))
                            elif si + 1 < len(scs):
                                sq_next = issue_head(pending[0], pending[1],
                                                     slice(0, CHUNK))

                    psP = pp41.tile([NP41, CHUNK], f32, tag="p41")
                    for j, t_i in enumerate(range(4)):
                        nc.tensor.matmul(psP[:, :], g1t[t_i][:, :],
                                         zt[t_i][:, :],
                                         start=(j == 0), stop=(j == 3))
                    zw = zwp.tile([NP41, CHUNK], f32r, tag="zw")
                    nc.vector.scalar_tensor_tensor(zw[:, :], psP[:, :], 1.0,
                                                   wt41[:, sl], MUL, MUL)
                    psO = po.tile([4, CHUNK], f32, tag="psO")
                    nc.tensor.matmul(psO[:, :], g2[:, :], zw[:, :],
                                     start=True, stop=True)
                    flush = (sc * (SCH // C), SCH // C) if cc == SUP - 1 else None
                    pending_out = (psO, outB, csl, flush)

            p_psO, p_outB, p_csl, p_flush = pending_out
            nc.scalar.copy(p_outB[:, p_csl], p_psO[:, :])
            p_b0, p_nb = p_flush
            nc.sync.dma_start(
                out_d[p_b0:p_b0 + p_nb].rearrange("b f c -> f b c"),
                p_outB[:, :])
    nc.compile()
    return nc


_NC_CACHE = {}


def _get_nc(bpc=BPC, repeat=1):
    key = (bpc, repeat)
    if key not in _NC_CACHE:
        _NC_CACHE[key] = build_nc(bpc, repeat)
    return _NC_CACHE[key]


def make_in_maps(inputs, bpc=BPC, ncores=NCORES):
    import ml_dtypes
    a_i = np.ascontiguousarray(inputs["a_i"], dtype=np.float32)
    y = np.ascontiguousarray(inputs["node_attrs"], dtype=np.float32)
    consts = _build_consts(
        np.asarray(inputs["U3_l0"], np.float32), np.asarray(inputs["U2_l0"], np.float32),
        np.asarray(inputs["U1_l0"], np.float32), np.asarray(inputs["U3_l1"], np.float32),
        np.asarray(inputs["U2_l1"], np.float32), np.asarray(inputs["U1_l1"], np.float32))
    Wmap = np.concatenate([
        np.asarray(inputs["W3_l0"], np.float32), np.asarray(inputs["W2_l0"], np.float32),
        np.asarray(inputs["W1_l0"], np.float32), np.asarray(inputs["W3_l1"], np.float32),
        np.asarray(inputs["W2_l1"], np.float32), np.asarray(inputs["W1_l1"], np.float32)],
        axis=1)                                    # [E, 19, C]
    cf = np.zeros((NOFF, 2 * NCOLS_S1 + NOFF + 4 + 4 * NP41), np.float32)
    cf[:, 0:NCOLS_S1] = consts["M1a"]
    cf[0:NXIN, NCOLS_S1:2 * NCOLS_S1] = consts["M1bc"]
    cg0 = 2 * NCOLS_S1 + NOFF + 4
    cf[0:NP41, 2 * NCOLS_S1 + NOFF:cg0] = consts["G2"]
    for t in range(4):
        r0, r1 = MCHUNKS[t]
        cf[0:r1 - r0, cg0 + NP41 * t:cg0 + NP41 * (t + 1)] = consts["G1"][r0:r1]
    cf[0:17, 2 * NCOLS_S1:2 * NCOLS_S1 + NOFF] = consts["SelAB"]
    shared = {"CF": cf,
              "ZfillC": np.zeros((16, CHUNK * 4), np.float32)}
    in_maps = []
    for core in range(ncores):
        b0 = core * bpc
        asl = a_i[b0:b0 + bpc]
        xa = np.empty((17, bpc * C), np.float32)
        xa[:16] = asl.transpose(2, 0, 1).reshape(DIM_I, bpc * C)
        xa[16] = 1.0
        w = y[b0:b0 + bpc] @ Wmap.reshape(E, NW * C)   # [bpc, NW*C]
        w41 = w.reshape(bpc, NW, C)[:, KROW, :]        # [bpc, 41, C]
        m = dict(shared)
        m["xa"] = xa
        m["Wt41"] = np.ascontiguousarray(
            w41.transpose(1, 0, 2).reshape(NP41, bpc * C).astype(ml_dtypes.bfloat16))
        in_maps.append(m)
    return in_maps


def assemble_output(results, bpc=BPC):
    outs = []
    for r in results:
        o = r["out"]
        outs.append(np.concatenate(
            [o[:, 0, :], o[:, 1:4, :].transpose(0, 2, 1).reshape(bpc, 3 * C)],
            axis=1))
    return np.concatenate(outs, axis=0)


def kernel(**inputs):
    from concourse import bass_utils
    nc = _get_nc()
    in_maps = make_in_maps(inputs)
    res = bass_utils.run_bass_kernel_spmd(nc, in_maps, core_ids=list(range(NCORES)))
    return assemble_output(res.results)


# revision 4
# speedup vs baseline: 1.0157x; 1.0157x over previous
"""Trainium2 Bass kernel for the MACE-style SymmetricContraction MessageBlock.

Math (per sample s=(b,c), x = a_i[b,c,:] in R^16, w*[b,k,c] = node_attrs @ W*):
  out0[b,c]   = sum U3_l0[i1,i2,i3,k] x_i1 x_i2 x_i3 w3[k]
              + sum U2_l0[i1,i2,k]    x_i1 x_i2      w2[k]
              + sum U1_l0[i1,k]       x_i1           w1[k]
  out1[b,c,l] = same with the l1 U/W tensors.

Device pipeline per 512-sample chunk. Pair monomials x_a*x_b come from the
sum-of-squares identity x_a x_b = ((x_a+x_b)^2 - x_a^2 - x_b^2)/2; the x_a^2
corrections are folded into the coefficient rows on the host:
  sq   = square(SelAB.T @ x)                 # 120 rows (PE+ACT)
  xin  = [x ; 1 ; x^2]                       # 33 rows; x^2 by ACT per chunk
  S1   = M1a'.T @ sq + M1bc'.T @ xin         # 431 cols, 4 m-chunks of <=119
  Z    = S1 * Xrep (bf16)                    # Xrep = x_{i1(col)} via bf16
                                             #   broadcast SBUF->SBUF DMA
  P41  = G1.T @ Z                            # reduce over i1 -> 41 cols
  ZW   = P41 * wt41                          # wt41 pre-expanded at setup
  out4 = G2.T @ ZW                           # out0, out1 l=0..2

Engine split per chunk: PE 14 matmul passes; ACT 2 squares + out copy;
DVE 4 Z mults + ZW; DMA streams x and broadcasts Xrep (f32r).

Sharding: data-parallel over nodes, 128 nodes per core on 8 cores.
"""
import numpy as np

B, C, DIM_I, E = 1024, 128, 16, 10
NCORES = 8
BPC = B // NCORES
S_PER_CORE = BPC * C
CHUNK = 512

OFFDIAG = [(a, b) for a in range(DIM_I) for b in range(a + 1, DIM_I)]  # 120
NOFF = len(OFFDIAG)
NC_CUBIC = 26
NCOLS_C = DIM_I * NC_CUBIC          # 416
NCOLS_S1 = NCOLS_C + 11 + 4         # 431
NP41 = 41
NW = 19
# m-chunks of 4 i1-groups (4*26=104); last chunk also carries quad/lin cols
MCHUNKS = [(0, 104), (104, 208), (208, 312), (312, NCOLS_S1)]
NXIN = 49  # xin rows: 0..15 x^2, 16..31 zero, 32..47 x, 48 ones

# P41 col -> packed w row (w3_l0:0-4, w2_l0:5-6, w1_l0:7, w3_l1:8-14,
# w2_l1:15-17, w1_l1:18)
KROW = [0] * NP41
for _m in range(5):
    KROW[_m] = _m
for _l in range(3):
    for _k in range(7):
        KROW[5 + _l * 7 + _k] = 8 + _k
KROW[26], KROW[27] = 5, 6
for _l in range(3):
    for _k in range(3):
        KROW[28 + _l * 3 + _k] = 15 + _k
KROW[37] = 7
for _l in range(3):
    KROW[38 + _l] = 18


# ---------------------------------------------------------------- host consts
def _pair_coeff(U3_l0, U3_l1, U2_l0, U2_l1, a, b):
    """431-col coefficient row for the pair monomial x_a*x_b (a<=b)."""
    dup = a < b
    row = np.zeros(NCOLS_S1, np.float32)
    for i1 in range(DIM_I):
        c0 = i1 * NC_CUBIC
        row[c0:c0 + 5] = U3_l0[i1, a, b] + (U3_l0[i1, b, a] if dup else 0)
        for l in range(3):
            row[c0 + 5 + l * 7: c0 + 12 + l * 7] = (
                U3_l1[l, i1, a, b] + (U3_l1[l, i1, b, a] if dup else 0))
    row[416:418] = U2_l0[a, b] + (U2_l0[b, a] if dup else 0)
    for l in range(3):
        row[418 + l * 3: 421 + l * 3] = (
            U2_l1[l, a, b] + (U2_l1[l, b, a] if dup else 0))
    return row


def _build_consts(U3_l0, U2_l0, U1_l0, U3_l1, U2_l1, U1_l1):
    M1a_raw = np.zeros((NOFF, NCOLS_S1), np.float32)
    for p, (a, b) in enumerate(OFFDIAG):
        M1a_raw[p] = _pair_coeff(U3_l0, U3_l1, U2_l0, U2_l1, a, b)
    # sum-of-squares substitution: sq rows carry M1a/2; diagonal rows get
    # the -1/2 sum of all off-diag rows touching that index
    M1a = 0.5 * M1a_raw
    M1bc = np.zeros((NXIN, NCOLS_S1), np.float32)
    for i in range(DIM_I):
        corr = np.zeros(NCOLS_S1, np.float32)
        for p, (a, b) in enumerate(OFFDIAG):
            if a == i or b == i:
                corr += M1a_raw[p]
        # row i: x_i^2 coefficient
        M1bc[i] = _pair_coeff(U3_l0, U3_l1, U2_l0, U2_l1, i, i) - 0.5 * corr
    for i in range(DIM_I):
        # rows 32..47: linear x_i coefficients
        M1bc[32 + i, 427] = U1_l0[i, 0]
        for l in range(3):
            M1bc[32 + i, 428 + l] = U1_l1[l, i, 0]

    G1 = np.zeros((NCOLS_S1, NP41), np.float32)
    for i1 in range(DIM_I):
        for m in range(NC_CUBIC):
            G1[i1 * NC_CUBIC + m, m] = 1
    for j in range(11):
        G1[416 + j, 26 + j] = 1
    for j in range(4):
        G1[427 + j, 37 + j] = 1

    G2 = np.zeros((NP41, 4), np.float32)
    G2[0:5, 0] = 1
    G2[26:28, 0] = 1
    G2[37, 0] = 1
    for l in range(3):
        G2[5 + l * 7: 12 + l * 7, 1 + l] = 1
        G2[28 + l * 3: 31 + l * 3, 1 + l] = 1
        G2[38 + l, 1 + l] = 1

    SelAB = np.zeros((17, NOFF), np.float32)
    for p, (a, b) in enumerate(OFFDIAG):
        SelAB[a, p] += 1
        SelAB[b, p] += 1
    return dict(M1a=M1a, M1bc=M1bc, G1=G1, G2=G2, SelAB=SelAB)


# ---------------------------------------------------------------- bass program
def build_nc(bpc=BPC, repeat=1):
    import concourse.bass as bass
    import concourse.bacc as bacc
    import concourse.mybir as mybir
    import concourse.tile as tile

    s_core = bpc * C
    nchunk = s_core // CHUNK
    f32 = mybir.dt.float32
    f32r = mybir.dt.float32r
    bf16 = mybir.dt.bfloat16
    MUL = mybir.AluOpType.mult

    nc = bacc.Bacc("TRN2", target_bir_lowering=False, debug=False)

    xa_d = nc.dram_tensor("xa", [17, s_core], f32r, kind="ExternalInput")
    wt41_d = nc.dram_tensor("Wt41", [NP41, s_core], bf16, kind="ExternalInput")
    # packed f32r consts: m1a | m1bc | sab | g2 along the free dim
    cf_d = nc.dram_tensor("CF", [NOFF, 2 * NCOLS_S1 + NOFF + 4 + 4 * NP41],
                          f32r, kind="ExternalInput")
    zf_d = nc.dram_tensor("ZfillC", [16, CHUNK * 4], f32r, kind="ExternalInput")
    out_d = nc.dram_tensor("out", [bpc, 4, C], f32, kind="ExternalOutput")

    SUP = 4
    SCH = CHUNK * SUP

    with tile.TileContext(nc) as tc:
        with (
            tc.tile_pool(name="const", bufs=1) as cp,
            tc.tile_pool(name="xbfp", bufs=3) as xbfp,
            tc.tile_pool(name="sqp", bufs=4) as sqp,
            tc.tile_pool(name="s1cp", bufs=3) as s1cp,
            tc.tile_pool(name="xrp", bufs=2) as xrp,
            tc.tile_pool(name="zp", bufs=8) as zp,
            tc.tile_pool(name="zwp", bufs=3) as zwp,
            tc.tile_pool(name="obp", bufs=3) as obp,
            tc.tile_pool(name="pab", bufs=1, space="PSUM") as pab,
            tc.tile_pool(name="ps1", bufs=3, space="PSUM") as ps1,
            tc.tile_pool(name="pp41", bufs=3, space="PSUM") as pp41,
            tc.tile_pool(name="po", bufs=1, space="PSUM") as po,
        ):
            def ctile(name, shape, dram, dt=f32r):
                t = cp.tile(shape, dt, tag=name)
                nc.sync.dma_start(t[:, :], dram[:])
                return t

            cf = ctile("cf", [NOFF, 2 * NCOLS_S1 + NOFF + 4 + 4 * NP41], cf_d)
            m1a = cf[:, 0:NCOLS_S1]
            m1bc = cf[0:NXIN, NCOLS_S1:2 * NCOLS_S1]

            g2 = cf[0:NP41, 2 * NCOLS_S1 + NOFF:2 * NCOLS_S1 + NOFF + 4]
            sab = cf[0:17, 2 * NCOLS_S1:2 * NCOLS_S1 + NOFF]
            CG0 = 2 * NCOLS_S1 + NOFF + 4
            g1t = [cf[0:MCHUNKS[t][1] - MCHUNKS[t][0],
                      CG0 + NP41 * t:CG0 + NP41 * (t + 1)] for t in range(4)]

            # per-node path weights, precomputed on the host. DMA issued
            # after the first superchunk's input streams (ZW needs it last).
            wt41 = cp.tile([NP41, s_core], bf16, tag="wt41")

            # xin ring: [x^2 ; 0 ; x ; 1] per superchunk. Manual ring so the
            # zero band (read by M1bc rows 16:32, which are all-zero coeffs)
            # is written once; engine writes stay quadrant-aligned.
            xin_ring = []
            for i in range(3):
                xt = cp.tile([NXIN, SCH], f32r, tag=f"xin{i}")
                nc.scalar.dma_start(xt[16:32, :], zf_d[:])
                xin_ring.append(xt)

            def issue_inputs(sc):
                """Stream xin/xbf + Xrep broadcasts for superchunk sc."""
                ssl = slice(SCH * sc, SCH * (sc + 1))
                xin = xin_ring[issue_inputs.n % 3]
                issue_inputs.n += 1
                nc.sync.dma_start(xin[32:49, :], xa_d[:, ssl])
                xbf = xbfp.tile([17, SCH], f32r, tag="xbf")
                nc.scalar.dma_start(xbf[:, :], xa_d[:, ssl])
                xr = xrp.tile([119, 4 * SCH], f32r, tag="xr")
                for t_i in range(4):
                    base = xbf[4 * t_i:4 * t_i + 4, :]
                    srcap = bass.AP(tensor=base.tensor, offset=base.offset,
                                    ap=[list(base.ap[0]), [0, NC_CUBIC],
                                        list(base.ap[-1])])
                    eng = nc.sync if t_i % 2 == 0 else nc.scalar
                    eng.dma_start(xr[0:104, SCH * t_i:SCH * (t_i + 1)], srcap)
                ones = xbf[16:17, :]
                onesap = bass.AP(tensor=ones.tensor, offset=ones.offset,
                                 ap=[list(ones.ap[0]), [0, 15],
                                     list(ones.ap[-1])])
                nc.scalar.dma_start(xr[104:119, 3 * SCH:4 * SCH], onesap)
                xrt = [xr[0:MCHUNKS[t][1] - MCHUNKS[t][0],
                          SCH * t:SCH * (t + 1)] for t in range(4)]
                return xin, xbf, xrt

            def issue_head(xin_t, xbf_t, csl_t):
                """Chunk head: pair sums (PE), x^2 rows and sq (ACT). x^2 is
                squared from the bf16 x copy: it lives on partitions 0:16,
                matching the destination (engines cannot shift partitions).
                Issued one chunk ahead of the S1 stage that uses it."""
                psAB = pab.tile([NOFF, CHUNK], f32, tag="psAB")
                nc.tensor.matmul(psAB[:, :], sab[:, :], xbf_t[0:17, csl_t],
                                 start=True, stop=True)
                nc.gpsimd.tensor_tensor(xin_t[0:16, csl_t],
                                        xbf_t[0:16, csl_t],
                                        xbf_t[0:16, csl_t], MUL)
                sq = sqp.tile([NOFF, CHUNK], f32r, tag="sq")
                nc.scalar.square(sq[:, :], psAB[:, :])
                return sq

            # ---- main loop over superchunks of SUP chunks. Input streams
            # are issued one superchunk ahead (so they never queue behind
            # output DMAs) and chunk heads one chunk ahead (so sq/x^2 are
            # ready when S1 runs).
            scs = [s for _ in range(repeat) for s in range(nchunk // SUP)]
            issue_inputs.n = 0
            pending = issue_inputs(scs[0])
            nc.sync.dma_start(wt41[:, :], wt41_d[:])
            sq_next = issue_head(pending[0], pending[1], slice(0, CHUNK))
            pending_out = None
            pending_zw = None
            for si, sc in enumerate(scs):
                xin, xbf, xrt = pending
                if si + 1 < len(scs):
                    pending = issue_inputs(scs[si + 1])
                outB = obp.tile([4, SCH], f32, tag="outB")

                for cc in range(SUP):
                    ch = SUP * sc + cc
                    sl = slice(CHUNK * ch, CHUNK * (ch + 1))
                    csl = slice(CHUNK * cc, CHUNK * (cc + 1))
                    sq = sq_next

                    # previous chunk's output copy + (at superchunk ends) its
                    # output DMA — pipelined here so the ACT queue never
                    # blocks on a not-yet-finished psO
                    if pending_out is not None:
                        p_psO, p_outB, p_csl, p_flush = pending_out
                        nc.scalar.copy(p_outB[:, p_csl], p_psO[:, :])
                        if p_flush is not None:
                            p_b0, p_nb = p_flush
                            nc.sync.dma_start(
                                out_d[p_b0:p_b0 + p_nb].rearrange(
                                    "b f c -> f b c"),
                                p_outB[:, :])

                    zt = []
                    for t_i, (c0, c1) in enumerate(MCHUNKS):
                        m = c1 - c0
                        psS = ps1.tile([m, CHUNK], f32, tag="s1")
                        nc.tensor.matmul(psS[:, :], m1a[:, c0:c1], sq[:, :],
                                         start=True, stop=False)
                        nc.tensor.matmul(psS[:, :], m1bc[:, c0:c1],
                                         xin[:, csl], start=False, stop=True)
                        z = zp.tile([m, CHUNK], f32r, tag="z")
                        nc.vector.scalar_tensor_tensor(
                            z[:, :], psS[:, :], 1.0, xrt[t_i][:, csl],
                            MUL, MUL)
                        zt.append(z)
                        if t_i == 1:
                            # head of the next chunk, early enough that its
                            # ACT squares land before that chunk's S1
                            if cc + 1 < SUP:
                                sq_next = issue_head(
                                    xin, xbf, slice(CHUNK * (cc + 1),
                                                    CHUNK * (cc + 2)))
                            elif si + 1 < len(scs):
                                sq_next = issue_head(pending[0], pending[1],
                                                     slice(0, CHUNK))

                    # previous chunk's ZW + psO, deferred so ZW never
                    # blocks the DVE queue head waiting on this chunk's P41
                    if pending_zw is not None:
                        p_psP, p_sl, p_outB, p_csl, p_flush = pending_zw
                        zw = zwp.tile([NP41, CHUNK], f32r, tag="zw")
                        nc.vector.scalar_tensor_tensor(
                            zw[:, :], p_psP[:, :], 1.0, wt41[:, p_sl],
                            MUL, MUL)
                        psO = po.tile([4, CHUNK], f32, tag="psO")
                        nc.tensor.matmul(psO[:, :], g2[:, :], zw[:, :],
                                         start=True, stop=True)
                        pending_out = (psO, p_outB, p_csl, p_flush)

                    psP = pp41.tile([NP41, CHUNK], f32, tag="p41")
                    for j, t_i in enumerate(range(4)):
                        nc.tensor.matmul(psP[:, :], g1t[t_i][:, :],
                                         zt[t_i][:, :],
                                         start=(j == 0), stop=(j == 3))
                    flush = (sc * (SCH // C), SCH // C) if cc == SUP - 1 else None
                    pending_zw = (psP, sl, outB, csl, flush)

            if pending_out is not None:
                p_psO, p_outB, p_csl, p_flush = pending_out
                nc.scalar.copy(p_outB[:, p_csl], p_psO[:, :])
                if p_flush is not None:
                    p_b0, p_nb = p_flush
                    nc.sync.dma_start(
                        out_d[p_b0:p_b0 + p_nb].rearrange("b f c -> f b c"),
                        p_outB[:, :])
            p_psP, p_sl, p_outB, p_csl, p_flush = pending_zw
            zw = zwp.tile([NP41, CHUNK], f32r, tag="zw")
            nc.vector.scalar_tensor_tensor(zw[:, :], p_psP[:, :], 1.0,
                                           wt41[:, p_sl], MUL, MUL)
            psO = po.tile([4, CHUNK], f32, tag="psO")
            nc.tensor.matmul(psO[:, :], g2[:, :], zw[:, :],
                             start=True, stop=True)
            nc.scalar.copy(p_outB[:, p_csl], psO[:, :])
            p_b0, p_nb = p_flush
            nc.sync.dma_start(
                out_d[p_b0:p_b0 + p_nb].rearrange("b f c -> f b c"),
                p_outB[:, :])
    nc.compile()
    return nc


_NC_CACHE = {}


def _get_nc(bpc=BPC, repeat=1):
    key = (bpc, repeat)
    if key not in _NC_CACHE:
        _NC_CACHE[key] = build_nc(bpc, repeat)
    return _NC_CACHE[key]


def make_in_maps(inputs, bpc=BPC, ncores=NCORES):
    import ml_dtypes
    a_i = np.ascontiguousarray(inputs["a_i"], dtype=np.float32)
    y = np.ascontiguousarray(inputs["node_attrs"], dtype=np.float32)
    consts = _build_consts(
        np.asarray(inputs["U3_l0"], np.float32), np.asarray(inputs["U2_l0"], np.float32),
        np.asarray(inputs["U1_l0"], np.float32), np.asarray(inputs["U3_l1"], np.float32),
        np.asarray(inputs["U2_l1"], np.float32), np.asarray(inputs["U1_l1"], np.float32))
    Wmap = np.concatenate([
        np.asarray(inputs["W3_l0"], np.float32), np.asarray(inputs["W2_l0"], np.float32),
        np.asarray(inputs["W1_l0"], np.float32), np.asarray(inputs["W3_l1"], np.float32),
        np.asarray(inputs["W2_l1"], np.float32), np.asarray(inputs["W1_l1"], np.float32)],
        axis=1)                                    # [E, 19, C]
    cf = np.zeros((NOFF, 2 * NCOLS_S1 + NOFF + 4 + 4 * NP41), np.float32)
    cf[:, 0:NCOLS_S1] = consts["M1a"]
    cf[0:NXIN, NCOLS_S1:2 * NCOLS_S1] = consts["M1bc"]
    cg0 = 2 * NCOLS_S1 + NOFF + 4
    cf[0:NP41, 2 * NCOLS_S1 + NOFF:cg0] = consts["G2"]
    for t in range(4):
        r0, r1 = MCHUNKS[t]
        cf[0:r1 - r0, cg0 + NP41 * t:cg0 + NP41 * (t + 1)] = consts["G1"][r0:r1]
    cf[0:17, 2 * NCOLS_S1:2 * NCOLS_S1 + NOFF] = consts["SelAB"]
    shared = {"CF": cf,
              "ZfillC": np.zeros((16, CHUNK * 4), np.float32)}
    in_maps = []
    for core in range(ncores):
        b0 = core * bpc
        asl = a_i[b0:b0 + bpc]
        xa = np.empty((17, bpc * C), np.float32)
        xa[:16] = asl.transpose(2, 0, 1).reshape(DIM_I, bpc * C)
        xa[16] = 1.0
        w = y[b0:b0 + bpc] @ Wmap.reshape(E, NW * C)   # [bpc, NW*C]
        w41 = w.reshape(bpc, NW, C)[:, KROW, :]        # [bpc, 41, C]
        m = dict(shared)
        m["xa"] = xa
        m["Wt41"] = np.ascontiguousarray(
            w41.transpose(1, 0, 2).reshape(NP41, bpc * C).astype(ml_dtypes.bfloat16))
        in_maps.append(m)
    return in_maps


def assemble_output(results, bpc=BPC):
    outs = []
    for r in results:
        o = r["out"]
        outs.append(np.concatenate(
            [o[:, 0, :], o[:, 1:4, :].transpose(0, 2, 1).reshape(bpc, 3 * C)],
            axis=1))
    return np.concatenate(outs, axis=0)


def kernel(**inputs):
    from concourse import bass_utils
    nc = _get_nc()
    in_maps = make_in_maps(inputs)
    res = bass_utils.run_bass_kernel_spmd(nc, in_maps, core_ids=list(range(NCORES)))
    return assemble_output(res.results)


# revision 6
# speedup vs baseline: 1.0343x; 1.0183x over previous
"""Trainium2 Bass kernel for the MACE-style SymmetricContraction MessageBlock.

Math (per sample s=(b,c), x = a_i[b,c,:] in R^16, w*[b,k,c] = node_attrs @ W*):
  out0[b,c]   = sum U3_l0[i1,i2,i3,k] x_i1 x_i2 x_i3 w3[k]
              + sum U2_l0[i1,i2,k]    x_i1 x_i2      w2[k]
              + sum U1_l0[i1,k]       x_i1           w1[k]
  out1[b,c,l] = same with the l1 U/W tensors.

Device pipeline per 512-sample chunk. Pair monomials x_a*x_b come from the
sum-of-squares identity x_a x_b = ((x_a+x_b)^2 - x_a^2 - x_b^2)/2; the x_a^2
corrections are folded into the coefficient rows on the host:
  sq   = square(SelAB.T @ x)                 # 120 rows (PE+ACT)
  xin  = [x^2 ; 0 ; x ; 1]                   # 49 rows; x^2 by Pool per chunk
  S1   = M1a'.T @ sq + M1bc'.T @ xin         # 431 cols, 4 m-chunks of <=119
  Z    = S1 * Xrep (f32r)                    # Xrep = x_{i1(col)} via
                                             #   broadcast SBUF->SBUF DMA
  P41  = G1.T @ Z                            # reduce over i1 -> 41 cols
  ZW   = P41 * wt41                          # wt41 pre-expanded at setup
  out4 = G2.T @ ZW                           # out0, out1 l=0..2

Engine split per chunk: PE 14 matmul passes; ACT 2 squares + out copy;
DVE 4 Z mults + ZW; DMA streams x and broadcasts Xrep (f32r).

Sharding: data-parallel over nodes, 128 nodes per core on 8 cores.
"""
import numpy as np

B, C, DIM_I, E = 1024, 128, 16, 10
NCORES = 8
BPC = B // NCORES
S_PER_CORE = BPC * C
CHUNK = 512

OFFDIAG = [(a, b) for a in range(DIM_I) for b in range(a + 1, DIM_I)]  # 120
NOFF = len(OFFDIAG)
NC_CUBIC = 26
NCOLS_C = DIM_I * NC_CUBIC          # 416
NCOLS_S1 = NCOLS_C + 11 + 4         # 431
NP41 = 41
NW = 19
# m-chunks of 4 i1-groups (4*26=104); last chunk also carries quad/lin cols
MCHUNKS = [(0, 104), (104, 208), (208, 312), (312, NCOLS_S1)]
NXIN = 49  # xin rows: 0..15 x^2, 16..31 zero, 32..47 x, 48 ones

# P41 col -> packed w row (w3_l0:0-4, w2_l0:5-6, w1_l0:7, w3_l1:8-14,
# w2_l1:15-17, w1_l1:18)
KROW = [0] * NP41
for _m in range(5):
    KROW[_m] = _m
for _l in range(3):
    for _k in range(7):
        KROW[5 + _l * 7 + _k] = 8 + _k
KROW[26], KROW[27] = 5, 6
for _l in range(3):
    for _k in range(3):
        KROW[28 + _l * 3 + _k] = 15 + _k
KROW[37] = 7
for _l in range(3):
    KROW[38 + _l] = 18


# ---------------------------------------------------------------- host consts
def _pair_coeff(U3_l0, U3_l1, U2_l0, U2_l1, a, b):
    """431-col coefficient row for the pair monomial x_a*x_b (a<=b)."""
    dup = a < b
    row = np.zeros(NCOLS_S1, np.float32)
    for i1 in range(DIM_I):
        c0 = i1 * NC_CUBIC
        row[c0:c0 + 5] = U3_l0[i1, a, b] + (U3_l0[i1, b, a] if dup else 0)
        for l in range(3):
            row[c0 + 5 + l * 7: c0 + 12 + l * 7] = (
                U3_l1[l, i1, a, b] + (U3_l1[l, i1, b, a] if dup else 0))
    row[416:418] = U2_l0[a, b] + (U2_l0[b, a] if dup else 0)
    for l in range(3):
        row[418 + l * 3: 421 + l * 3] = (
            U2_l1[l, a, b] + (U2_l1[l, b, a] if dup else 0))
    return row


def _build_consts(U3_l0, U2_l0, U1_l0, U3_l1, U2_l1, U1_l1):
    M1a_raw = np.zeros((NOFF, NCOLS_S1), np.float32)
    for p, (a, b) in enumerate(OFFDIAG):
        M1a_raw[p] = _pair_coeff(U3_l0, U3_l1, U2_l0, U2_l1, a, b)
    # sum-of-squares substitution: sq rows carry M1a/2; diagonal rows get
    # the -1/2 sum of all off-diag rows touching that index
    M1a = 0.5 * M1a_raw
    M1bc = np.zeros((NXIN, NCOLS_S1), np.float32)
    for i in range(DIM_I):
        corr = np.zeros(NCOLS_S1, np.float32)
        for p, (a, b) in enumerate(OFFDIAG):
            if a == i or b == i:
                corr += M1a_raw[p]
        # row i: x_i^2 coefficient
        M1bc[i] = _pair_coeff(U3_l0, U3_l1, U2_l0, U2_l1, i, i) - 0.5 * corr
    for i in range(DIM_I):
        # rows 32..47: linear x_i coefficients
        M1bc[32 + i, 427] = U1_l0[i, 0]
        for l in range(3):
            M1bc[32 + i, 428 + l] = U1_l1[l, i, 0]

    G1 = np.zeros((NCOLS_S1, NP41), np.float32)
    for i1 in range(DIM_I):
        for m in range(NC_CUBIC):
            G1[i1 * NC_CUBIC + m, m] = 1
    for j in range(11):
        G1[416 + j, 26 + j] = 1
    for j in range(4):
        G1[427 + j, 37 + j] = 1

    G2 = np.zeros((NP41, 4), np.float32)
    G2[0:5, 0] = 1
    G2[26:28, 0] = 1
    G2[37, 0] = 1
    for l in range(3):
        G2[5 + l * 7: 12 + l * 7, 1 + l] = 1
        G2[28 + l * 3: 31 + l * 3, 1 + l] = 1
        G2[38 + l, 1 + l] = 1

    SelAB = np.zeros((17, NOFF), np.float32)
    for p, (a, b) in enumerate(OFFDIAG):
        SelAB[a, p] += 1
        SelAB[b, p] += 1
    return dict(M1a=M1a, M1bc=M1bc, G1=G1, G2=G2, SelAB=SelAB)


# ---------------------------------------------------------------- bass program
def build_nc(bpc=BPC, repeat=1):
    import concourse.bass as bass
    import concourse.bacc as bacc
    import concourse.mybir as mybir
    import concourse.tile as tile

    s_core = bpc * C
    nchunk = s_core // CHUNK
    f32 = mybir.dt.float32
    f32r = mybir.dt.float32r
    bf16 = mybir.dt.bfloat16
    MUL = mybir.AluOpType.mult

    nc = bacc.Bacc("TRN2", target_bir_lowering=False, debug=False)

    xa_d = nc.dram_tensor("xa", [17, s_core], f32r, kind="ExternalInput")
    wt41_d = nc.dram_tensor("Wt41", [NP41, s_core], bf16, kind="ExternalInput")
    # packed f32r consts: m1a | m1bc | sab | g2 along the free dim
    cf_d = nc.dram_tensor("CF", [NOFF, 2 * NCOLS_S1 + NOFF + 4 + 4 * NP41],
                          f32r, kind="ExternalInput")
    zf_d = nc.dram_tensor("ZfillC", [16, CHUNK * 4], f32r, kind="ExternalInput")
    out_d = nc.dram_tensor("out", [bpc, 4, C], f32, kind="ExternalOutput")

    SUP = 4
    SCH = CHUNK * SUP

    with tile.TileContext(nc) as tc:
        with (
            tc.tile_pool(name="const", bufs=1) as cp,
            tc.tile_pool(name="xbfp", bufs=3) as xbfp,
            tc.tile_pool(name="sqp", bufs=4) as sqp,
            tc.tile_pool(name="s1cp", bufs=3) as s1cp,
            tc.tile_pool(name="xrp", bufs=2) as xrp,
            tc.tile_pool(name="zp", bufs=8) as zp,
            tc.tile_pool(name="zwp", bufs=3) as zwp,
            tc.tile_pool(name="obp", bufs=3) as obp,
            tc.tile_pool(name="pab", bufs=1, space="PSUM") as pab,
            tc.tile_pool(name="ps1", bufs=3, space="PSUM") as ps1,
            tc.tile_pool(name="pp41", bufs=3, space="PSUM") as pp41,
            tc.tile_pool(name="po", bufs=1, space="PSUM") as po,
        ):
            def ctile(name, shape, dram, dt=f32r):
                t = cp.tile(shape, dt, tag=name)
                nc.sync.dma_start(t[:, :], dram[:])
                return t

            cf = ctile("cf", [NOFF, 2 * NCOLS_S1 + NOFF + 4 + 4 * NP41], cf_d)
            m1a = cf[:, 0:NCOLS_S1]
            m1bc = cf[0:NXIN, NCOLS_S1:2 * NCOLS_S1]

            g2 = cf[0:NP41, 2 * NCOLS_S1 + NOFF:2 * NCOLS_S1 + NOFF + 4]
            sab = cf[0:17, 2 * NCOLS_S1:2 * NCOLS_S1 + NOFF]
            CG0 = 2 * NCOLS_S1 + NOFF + 4
            g1t = [cf[0:MCHUNKS[t][1] - MCHUNKS[t][0],
                      CG0 + NP41 * t:CG0 + NP41 * (t + 1)] for t in range(4)]

            # per-node path weights, precomputed on the host. DMA issued
            # after the first superchunk's input streams (ZW needs it last).
            wt41 = cp.tile([NP41, s_core], bf16, tag="wt41")

            # xin ring: [x^2 ; 0 ; x ; 1] per superchunk. Manual ring so the
            # zero band (read by M1bc rows 16:32, which are all-zero coeffs)
            # is written once; engine writes stay quadrant-aligned.
            xin_ring = []
            for i in range(3):
                xt = cp.tile([NXIN, SCH], f32r, tag=f"xin{i}")
                xin_ring.append(xt)

            def issue_inputs(sc):
                """Stream xin/xbf + Xrep broadcasts for superchunk sc."""
                ssl = slice(SCH * sc, SCH * (sc + 1))
                xin = xin_ring[issue_inputs.n % 3]
                issue_inputs.n += 1
                nc.sync.dma_start(xin[32:49, :], xa_d[:, ssl])
                xbf = xbfp.tile([17, SCH], f32r, tag="xbf")
                nc.scalar.dma_start(xbf[:, :], xa_d[:, ssl])
                xr = xrp.tile([119, 4 * SCH], f32r, tag="xr")
                first = issue_inputs.n == 1
                for t_i in range(4):
                    base = xbf[4 * t_i:4 * t_i + 4, :]
                    eng = nc.sync if t_i % 2 == 0 else nc.scalar
                    if first and t_i == 0:
                        # startup: chunk-sized pieces so chunk 0's Z starts
                        # after ~1/16 of the broadcast instead of all of it
                        for q in range(4):
                            b2 = base[:, CHUNK * q:CHUNK * (q + 1)]
                            srcap = bass.AP(
                                tensor=b2.tensor, offset=b2.offset,
                                ap=[list(b2.ap[0]), [0, NC_CUBIC],
                                    list(b2.ap[-1])])
                            eng.dma_start(
                                xr[0:104, SCH * t_i + CHUNK * q:
                                   SCH * t_i + CHUNK * (q + 1)], srcap)
                        continue
                    srcap = bass.AP(tensor=base.tensor, offset=base.offset,
                                    ap=[list(base.ap[0]), [0, NC_CUBIC],
                                        list(base.ap[-1])])
                    eng.dma_start(xr[0:104, SCH * t_i:SCH * (t_i + 1)], srcap)
                ones = xbf[16:17, :]
                onesap = bass.AP(tensor=ones.tensor, offset=ones.offset,
                                 ap=[list(ones.ap[0]), [0, 15],
                                     list(ones.ap[-1])])
                nc.scalar.dma_start(xr[104:119, 3 * SCH:4 * SCH], onesap)
                xrt = [xr[0:MCHUNKS[t][1] - MCHUNKS[t][0],
                          SCH * t:SCH * (t + 1)] for t in range(4)]
                return xin, xbf, xrt

            def issue_head(xin_t, xbf_t, csl_t):
                """Chunk head: pair sums (PE), x^2 rows and sq (ACT). x^2 is
                squared from the bf16 x copy: it lives on partitions 0:16,
                matching the destination (engines cannot shift partitions).
                Issued one chunk ahead of the S1 stage that uses it."""
                psAB = pab.tile([NOFF, CHUNK], f32, tag="psAB")
                nc.tensor.matmul(psAB[:, :], sab[:, :], xbf_t[0:17, csl_t],
                                 start=True, stop=True)
                nc.gpsimd.tensor_tensor(xin_t[0:16, csl_t],
                                        xbf_t[0:16, csl_t],
                                        xbf_t[0:16, csl_t], MUL)
                sq = sqp.tile([NOFF, CHUNK], f32r, tag="sq")
                nc.scalar.square(sq[:, :], psAB[:, :])
                return sq

            # ---- main loop over superchunks of SUP chunks. Input streams
            # are issued one superchunk ahead (so they never queue behind
            # output DMAs) and chunk heads one chunk ahead (so sq/x^2 are
            # ready when S1 runs).
            scs = [s for _ in range(repeat) for s in range(nchunk // SUP)]
            issue_inputs.n = 0
            pending = issue_inputs(scs[0])
            nc.sync.dma_start(wt41[:, :], wt41_d[:])
            for xt in xin_ring:
                nc.scalar.dma_start(xt[16:32, :], zf_d[:])
            sq_next = issue_head(pending[0], pending[1], slice(0, CHUNK))
            pending_out = None
            pending_zw = None
            for si, sc in enumerate(scs):
                xin, xbf, xrt = pending
                if si + 1 < len(scs):
                    pending = issue_inputs(scs[si + 1])
                outB = obp.tile([4, SCH], f32, tag="outB")

                for cc in range(SUP):
                    ch = SUP * sc + cc
                    sl = slice(CHUNK * ch, CHUNK * (ch + 1))
                    csl = slice(CHUNK * cc, CHUNK * (cc + 1))
                    sq = sq_next

                    # previous chunk's output copy + (at superchunk ends) its
                    # output DMA — pipelined here so the ACT queue never
                    # blocks on a not-yet-finished psO
                    if pending_out is not None:
                        p_psO, p_outB, p_csl, p_flush = pending_out
                        nc.scalar.copy(p_outB[:, p_csl], p_psO[:, :])
                        if p_flush is not None:
                            p_b0, p_nb = p_flush
                            nc.sync.dma_start(
                                out_d[p_b0:p_b0 + p_nb].rearrange(
                                    "b f c -> f b c"),
                                p_outB[:, :])

                    zt = []
                    for t_i, (c0, c1) in enumerate(MCHUNKS):
                        m = c1 - c0
                        psS = ps1.tile([m, CHUNK], f32, tag="s1")
                        nc.tensor.matmul(psS[:, :], m1a[:, c0:c1], sq[:, :],
                                         start=True, stop=False)
                        nc.tensor.matmul(psS[:, :], m1bc[:, c0:c1],
                                         xin[:, csl], start=False, stop=True)
                        z = zp.tile([m, CHUNK], f32r, tag="z")
                        nc.vector.scalar_tensor_tensor(
                            z[:, :], psS[:, :], 1.0, xrt[t_i][:, csl],
                            MUL, MUL)
                        zt.append(z)
                        if t_i == 1:
                            # head of the next chunk, early enough that its
                            # ACT squares land before that chunk's S1
                            if cc + 1 < SUP:
                                sq_next = issue_head(
                                    xin, xbf, slice(CHUNK * (cc + 1),
                                                    CHUNK * (cc + 2)))
                            elif si + 1 < len(scs):
                                sq_next = issue_head(pending[0], pending[1],
                                                     slice(0, CHUNK))

                    # previous chunk's ZW + psO, deferred so ZW never
                    # blocks the DVE queue head waiting on this chunk's P41
                    if pending_zw is not None:
                        p_psP, p_sl, p_outB, p_csl, p_flush = pending_zw
                        zw = zwp.tile([NP41, CHUNK], f32r, tag="zw")
                        nc.vector.scalar_tensor_tensor(
                            zw[:, :], p_psP[:, :], 1.0, wt41[:, p_sl],
                            MUL, MUL)
                        psO = po.tile([4, CHUNK], f32, tag="psO")
                        nc.tensor.matmul(psO[:, :], g2[:, :], zw[:, :],
                                         start=True, stop=True)
                        pending_out = (psO, p_outB, p_csl, p_flush)

                    psP = pp41.tile([NP41, CHUNK], f32, tag="p41")
                    for j, t_i in enumerate(range(4)):
                        nc.tensor.matmul(psP[:, :], g1t[t_i][:, :],
                                         zt[t_i][:, :],
                                         start=(j == 0), stop=(j == 3))
                    flush = (sc * (SCH // C), SCH // C) if cc == SUP - 1 else None
                    pending_zw = (psP, sl, outB, csl, flush)

            if pending_out is not None:
                p_psO, p_outB, p_csl, p_flush = pending_out
                nc.scalar.copy(p_outB[:, p_csl], p_psO[:, :])
                if p_flush is not None:
                    p_b0, p_nb = p_flush
                    nc.sync.dma_start(
                        out_d[p_b0:p_b0 + p_nb].rearrange("b f c -> f b c"),
                        p_outB[:, :])
            p_psP, p_sl, p_outB, p_csl, p_flush = pending_zw
            zw = zwp.tile([NP41, CHUNK], f32r, tag="zw")
            nc.vector.scalar_tensor_tensor(zw[:, :], p_psP[:, :], 1.0,
                                           wt41[:, p_sl], MUL, MUL)
            psO = po.tile([4, CHUNK], f32, tag="psO")
            nc.tensor.matmul(psO[:, :], g2[:, :], zw[:, :],
                             start=True, stop=True)
            nc.scalar.copy(p_outB[:, p_csl], psO[:, :])
            p_b0, p_nb = p_flush
            nc.sync.dma_start(
                out_d[p_b0:p_b0 + p_nb].rearrange("b f c -> f b c"),
                p_outB[:, :])
    nc.compile()
    return nc


_NC_CACHE = {}


def _get_nc(bpc=BPC, repeat=1):
    key = (bpc, repeat)
    if key not in _NC_CACHE:
        _NC_CACHE[key] = build_nc(bpc, repeat)
    return _NC_CACHE[key]


def make_in_maps(inputs, bpc=BPC, ncores=NCORES):
    import ml_dtypes
    a_i = np.ascontiguousarray(inputs["a_i"], dtype=np.float32)
    y = np.ascontiguousarray(inputs["node_attrs"], dtype=np.float32)
    consts = _build_consts(
        np.asarray(inputs["U3_l0"], np.float32), np.asarray(inputs["U2_l0"], np.float32),
        np.asarray(inputs["U1_l0"], np.float32), np.asarray(inputs["U3_l1"], np.float32),
        np.asarray(inputs["U2_l1"], np.float32), np.asarray(inputs["U1_l1"], np.float32))
    Wmap = np.concatenate([
        np.asarray(inputs["W3_l0"], np.float32), np.asarray(inputs["W2_l0"], np.float32),
        np.asarray(inputs["W1_l0"], np.float32), np.asarray(inputs["W3_l1"], np.float32),
        np.asarray(inputs["W2_l1"], np.float32), np.asarray(inputs["W1_l1"], np.float32)],
        axis=1)                                    # [E, 19, C]
    cf = np.zeros((NOFF, 2 * NCOLS_S1 + NOFF + 4 + 4 * NP41), np.float32)
    cf[:, 0:NCOLS_S1] = consts["M1a"]
    cf[0:NXIN, NCOLS_S1:2 * NCOLS_S1] = consts["M1bc"]
    cg0 = 2 * NCOLS_S1 + NOFF + 4
    cf[0:NP41, 2 * NCOLS_S1 + NOFF:cg0] = consts["G2"]
    for t in range(4):
        r0, r1 = MCHUNKS[t]
        cf[0:r1 - r0, cg0 + NP41 * t:cg0 + NP41 * (t + 1)] = consts["G1"][r0:r1]
    cf[0:17, 2 * NCOLS_S1:2 * NCOLS_S1 + NOFF] = consts["SelAB"]
    shared = {"CF": cf,
              "ZfillC": np.zeros((16, CHUNK * 4), np.float32)}
    in_maps = []
    for core in range(ncores):
        b0 = core * bpc
        asl = a_i[b0:b0 + bpc]
        xa = np.empty((17, bpc * C), np.float32)
        xa[:16] = asl.transpose(2, 0, 1).reshape(DIM_I, bpc * C)
        xa[16] = 1.0
        w = y[b0:b0 + bpc] @ Wmap.reshape(E, NW * C)   # [bpc, NW*C]
        w41 = w.reshape(bpc, NW, C)[:, KROW, :]        # [bpc, 41, C]
        m = dict(shared)
        m["xa"] = xa
        m["Wt41"] = np.ascontiguousarray(
            w41.transpose(1, 0, 2).reshape(NP41, bpc * C).astype(ml_dtypes.bfloat16))
        in_maps.append(m)
    return in_maps


def assemble_output(results, bpc=BPC):
    outs = []
    for r in results:
        o = r["out"]
        outs.append(np.concatenate(
            [o[:, 0, :], o[:, 1:4, :].transpose(0, 2, 1).reshape(bpc, 3 * C)],
            axis=1))
    return np.concatenate(outs, axis=0)


def kernel(**inputs):
    from concourse import bass_utils
    nc = _get_nc()
    in_maps = make_in_maps(inputs)
    res = bass_utils.run_bass_kernel_spmd(nc, in_maps, core_ids=list(range(NCORES)))
    return assemble_output(res.results)


# revision 7
# speedup vs baseline: 1.0576x; 1.0226x over previous
"""Trainium2 Bass kernel for the MACE-style SymmetricContraction MessageBlock.

Math (per sample s=(b,c), x = a_i[b,c,:] in R^16, w*[b,k,c] = node_attrs @ W*):
  out0[b,c]   = sum U3_l0[i1,i2,i3,k] x_i1 x_i2 x_i3 w3[k]
              + sum U2_l0[i1,i2,k]    x_i1 x_i2      w2[k]
              + sum U1_l0[i1,k]       x_i1           w1[k]
  out1[b,c,l] = same with the l1 U/W tensors.

Device pipeline per 512-sample chunk. Pair monomials x_a*x_b come from the
sum-of-squares identity x_a x_b = ((x_a+x_b)^2 - x_a^2 - x_b^2)/2; the x_a^2
corrections are folded into the coefficient rows on the host:
  sq   = square(SelAB.T @ x)                 # 120 rows (PE+ACT)
  xin  = [x^2 ; 0 ; x ; 1]                   # 49 rows; x^2 by Pool per chunk
  S1   = M1a'.T @ sq + M1bc'.T @ xin         # 431 cols, 4 m-chunks of <=119
  Z    = S1 * Xrep (f32r)                    # Xrep = x_{i1(col)} via
                                             #   broadcast SBUF->SBUF DMA
  P41  = G1.T @ Z                            # reduce over i1 -> 41 cols
  ZW   = P41 * wt41                          # wt41 pre-expanded at setup
  out4 = G2.T @ ZW                           # out0, out1 l=0..2

Engine split per chunk: PE 14 matmul passes; ACT 2 squares + out copy;
DVE 4 Z mults + ZW; DMA streams x and broadcasts Xrep (f32r).

Sharding: data-parallel over nodes, 128 nodes per core on 8 cores.
"""
import numpy as np

B, C, DIM_I, E = 1024, 128, 16, 10
NCORES = 8
BPC = B // NCORES
S_PER_CORE = BPC * C
CHUNK = 512

OFFDIAG = [(a, b) for a in range(DIM_I) for b in range(a + 1, DIM_I)]  # 120
NOFF = len(OFFDIAG)
NC_CUBIC = 26
NCOLS_C = DIM_I * NC_CUBIC          # 416
NCOLS_S1 = NCOLS_C + 11 + 4         # 431
NP41 = 41
NW = 19
# m-chunks of 4 i1-groups (4*26=104); last chunk also carries quad/lin cols
MCHUNKS = [(0, 104), (104, 208), (208, 312), (312, NCOLS_S1)]
NXIN = 49  # xin rows: 0..15 x^2, 16..31 zero, 32..47 x, 48 ones

# P41 col -> packed w row (w3_l0:0-4, w2_l0:5-6, w1_l0:7, w3_l1:8-14,
# w2_l1:15-17, w1_l1:18)
KROW = [0] * NP41
for _m in range(5):
    KROW[_m] = _m
for _l in range(3):
    for _k in range(7):
        KROW[5 + _l * 7 + _k] = 8 + _k
KROW[26], KROW[27] = 5, 6
for _l in range(3):
    for _k in range(3):
        KROW[28 + _l * 3 + _k] = 15 + _k
KROW[37] = 7
for _l in range(3):
    KROW[38 + _l] = 18


# ---------------------------------------------------------------- host consts
def _pair_coeff(U3_l0, U3_l1, U2_l0, U2_l1, a, b):
    """431-col coefficient row for the pair monomial x_a*x_b (a<=b)."""
    dup = a < b
    row = np.zeros(NCOLS_S1, np.float32)
    for i1 in range(DIM_I):
        c0 = i1 * NC_CUBIC
        row[c0:c0 + 5] = U3_l0[i1, a, b] + (U3_l0[i1, b, a] if dup else 0)
        for l in range(3):
            row[c0 + 5 + l * 7: c0 + 12 + l * 7] = (
                U3_l1[l, i1, a, b] + (U3_l1[l, i1, b, a] if dup else 0))
    row[416:418] = U2_l0[a, b] + (U2_l0[b, a] if dup else 0)
    for l in range(3):
        row[418 + l * 3: 421 + l * 3] = (
            U2_l1[l, a, b] + (U2_l1[l, b, a] if dup else 0))
    return row


def _build_consts(U3_l0, U2_l0, U1_l0, U3_l1, U2_l1, U1_l1):
    M1a_raw = np.zeros((NOFF, NCOLS_S1), np.float32)
    for p, (a, b) in enumerate(OFFDIAG):
        M1a_raw[p] = _pair_coeff(U3_l0, U3_l1, U2_l0, U2_l1, a, b)
    # sum-of-squares substitution: sq rows carry M1a/2; diagonal rows get
    # the -1/2 sum of all off-diag rows touching that index
    M1a = 0.5 * M1a_raw
    M1bc = np.zeros((NXIN, NCOLS_S1), np.float32)
    for i in range(DIM_I):
        corr = np.zeros(NCOLS_S1, np.float32)
        for p, (a, b) in enumerate(OFFDIAG):
            if a == i or b == i:
                corr += M1a_raw[p]
        # row i: x_i^2 coefficient
        M1bc[i] = _pair_coeff(U3_l0, U3_l1, U2_l0, U2_l1, i, i) - 0.5 * corr
    for i in range(DIM_I):
        # rows 32..47: linear x_i coefficients
        M1bc[32 + i, 427] = U1_l0[i, 0]
        for l in range(3):
            M1bc[32 + i, 428 + l] = U1_l1[l, i, 0]

    G1 = np.zeros((NCOLS_S1, NP41), np.float32)
    for i1 in range(DIM_I):
        for m in range(NC_CUBIC):
            G1[i1 * NC_CUBIC + m, m] = 1
    for j in range(11):
        G1[416 + j, 26 + j] = 1
    for j in range(4):
        G1[427 + j, 37 + j] = 1

    G2 = np.zeros((NP41, 4), np.float32)
    G2[0:5, 0] = 1
    G2[26:28, 0] = 1
    G2[37, 0] = 1
    for l in range(3):
        G2[5 + l * 7: 12 + l * 7, 1 + l] = 1
        G2[28 + l * 3: 31 + l * 3, 1 + l] = 1
        G2[38 + l, 1 + l] = 1

    SelAB = np.zeros((17, NOFF), np.float32)
    for p, (a, b) in enumerate(OFFDIAG):
        SelAB[a, p] += 1
        SelAB[b, p] += 1
    return dict(M1a=M1a, M1bc=M1bc, G1=G1, G2=G2, SelAB=SelAB)


# ---------------------------------------------------------------- bass program
def build_nc(bpc=BPC, repeat=1):
    import concourse.bass as bass
    import concourse.bacc as bacc
    import concourse.mybir as mybir
    import concourse.tile as tile

    s_core = bpc * C
    nchunk = s_core // CHUNK
    f32 = mybir.dt.float32
    f32r = mybir.dt.float32r
    bf16 = mybir.dt.bfloat16
    MUL = mybir.AluOpType.mult

    nc = bacc.Bacc("TRN2", target_bir_lowering=False, debug=False)

    xa_d = nc.dram_tensor("xa", [17, s_core], f32r, kind="ExternalInput")
    wt41_d = nc.dram_tensor("Wt41", [NP41, s_core], bf16, kind="ExternalInput")
    # packed f32r consts: m1a | m1bc | sab | g2 along the free dim
    cf_d = nc.dram_tensor("CF", [NOFF, 2 * NCOLS_S1 + NOFF + 4 + 4 * NP41],
                          f32r, kind="ExternalInput")
    zf_d = nc.dram_tensor("ZfillC", [16, CHUNK * 4], f32r, kind="ExternalInput")
    out_d = nc.dram_tensor("out", [bpc, 4, C], f32, kind="ExternalOutput")

    SUP = 4
    SCH = CHUNK * SUP

    with tile.TileContext(nc) as tc:
        with (
            tc.tile_pool(name="const", bufs=1) as cp,
            tc.tile_pool(name="xbfp", bufs=3) as xbfp,
            tc.tile_pool(name="sqp", bufs=4) as sqp,
            tc.tile_pool(name="s1cp", bufs=3) as s1cp,
            tc.tile_pool(name="xrp", bufs=2) as xrp,
            tc.tile_pool(name="zp", bufs=8) as zp,
            tc.tile_pool(name="zwp", bufs=3) as zwp,
            tc.tile_pool(name="obp", bufs=3) as obp,
            tc.tile_pool(name="pab", bufs=1, space="PSUM") as pab,
            tc.tile_pool(name="ps1", bufs=3, space="PSUM") as ps1,
            tc.tile_pool(name="pp41", bufs=3, space="PSUM") as pp41,
            tc.tile_pool(name="po", bufs=1, space="PSUM") as po,
        ):
            def ctile(name, shape, dram, dt=f32r):
                t = cp.tile(shape, dt, tag=name)
                nc.sync.dma_start(t[:, :], dram[:])
                return t

            cf = ctile("cf", [NOFF, 2 * NCOLS_S1 + NOFF + 4 + 4 * NP41], cf_d)
            m1a = cf[:, 0:NCOLS_S1]
            m1bc = cf[0:NXIN, NCOLS_S1:2 * NCOLS_S1]

            g2 = cf[0:NP41, 2 * NCOLS_S1 + NOFF:2 * NCOLS_S1 + NOFF + 4]
            sab = cf[0:17, 2 * NCOLS_S1:2 * NCOLS_S1 + NOFF]
            CG0 = 2 * NCOLS_S1 + NOFF + 4
            g1t = [cf[0:MCHUNKS[t][1] - MCHUNKS[t][0],
                      CG0 + NP41 * t:CG0 + NP41 * (t + 1)] for t in range(4)]

            # per-node path weights, precomputed on the host. DMA issued
            # after the first superchunk's input streams (ZW needs it last).
            wt41 = cp.tile([NP41, s_core], bf16, tag="wt41")

            # xin ring: [x^2 ; 0 ; x ; 1] per superchunk. Manual ring so the
            # zero band (read by M1bc rows 16:32, which are all-zero coeffs)
            # is written once; engine writes stay quadrant-aligned.
            xin_ring = []
            for i in range(3):
                xt = cp.tile([NXIN, SCH], f32r, tag=f"xin{i}")
                xin_ring.append(xt)

            def issue_inputs(sc):
                """Stream xin/xbf + Xrep broadcasts for superchunk sc."""
                ssl = slice(SCH * sc, SCH * (sc + 1))
                xin = xin_ring[issue_inputs.n % 3]
                issue_inputs.n += 1
                nc.sync.dma_start(xin[32:49, :], xa_d[:, ssl])
                xbf = xbfp.tile([17, SCH], f32r, tag="xbf")
                nc.scalar.dma_start(xbf[:, :], xa_d[:, ssl])
                xr = xrp.tile([119, 4 * SCH], f32r, tag="xr")
                first = issue_inputs.n == 1
                for t_i in range(4):
                    base = xbf[4 * t_i:4 * t_i + 4, :]
                    eng = nc.sync if t_i % 2 == 0 else nc.scalar
                    if first and t_i == 0:
                        # startup: chunk-sized pieces so chunk 0's Z starts
                        # after ~1/16 of the broadcast instead of all of it
                        for q in range(4):
                            b2 = base[:, CHUNK * q:CHUNK * (q + 1)]
                            srcap = bass.AP(
                                tensor=b2.tensor, offset=b2.offset,
                                ap=[list(b2.ap[0]), [0, NC_CUBIC],
                                    list(b2.ap[-1])])
                            eng.dma_start(
                                xr[0:104, SCH * t_i + CHUNK * q:
                                   SCH * t_i + CHUNK * (q + 1)], srcap)
                        continue
                    srcap = bass.AP(tensor=base.tensor, offset=base.offset,
                                    ap=[list(base.ap[0]), [0, NC_CUBIC],
                                        list(base.ap[-1])])
                    eng.dma_start(xr[0:104, SCH * t_i:SCH * (t_i + 1)], srcap)
                ones = xbf[16:17, :]
                onesap = bass.AP(tensor=ones.tensor, offset=ones.offset,
                                 ap=[list(ones.ap[0]), [0, 15],
                                     list(ones.ap[-1])])
                nc.scalar.dma_start(xr[104:119, 3 * SCH:4 * SCH], onesap)
                xrt = [xr[0:MCHUNKS[t][1] - MCHUNKS[t][0],
                          SCH * t:SCH * (t + 1)] for t in range(4)]
                return xin, xbf, xrt

            def issue_head(xin_t, xbf_t, csl_t):
                """Chunk head: pair sums (PE), x^2 rows and sq (ACT). x^2 is
                squared from the bf16 x copy: it lives on partitions 0:16,
                matching the destination (engines cannot shift partitions).
                Issued one chunk ahead of the S1 stage that uses it."""
                psAB = pab.tile([NOFF, CHUNK], f32, tag="psAB")
                nc.tensor.matmul(psAB[:, :], sab[:, :], xbf_t[0:17, csl_t],
                                 start=True, stop=True)
                nc.gpsimd.tensor_tensor(xin_t[0:16, csl_t],
                                        xbf_t[0:16, csl_t],
                                        xbf_t[0:16, csl_t], MUL)
                sq = sqp.tile([NOFF, CHUNK], f32r, tag="sq")
                nc.scalar.square(sq[:, :], psAB[:, :])
                return sq

            # ---- main loop over superchunks of SUP chunks. Input streams
            # are issued one superchunk ahead (so they never queue behind
            # output DMAs) and chunk heads one chunk ahead (so sq/x^2 are
            # ready when S1 runs).
            scs = [s for _ in range(repeat) for s in range(nchunk // SUP)]
            issue_inputs.n = 0
            pending = issue_inputs(scs[0])
            nc.sync.dma_start(wt41[:, :], wt41_d[:])
            for xt in xin_ring:
                nc.scalar.dma_start(xt[16:32, :], zf_d[:])
            sq_next = issue_head(pending[0], pending[1], slice(0, CHUNK))
            pending_out = None
            pending_zw = None
            for si, sc in enumerate(scs):
                xin, xbf, xrt = pending
                if si + 1 < len(scs):
                    pending = issue_inputs(scs[si + 1])
                outB = obp.tile([4, SCH], f32, tag="outB")

                for cc in range(SUP):
                    ch = SUP * sc + cc
                    sl = slice(CHUNK * ch, CHUNK * (ch + 1))
                    csl = slice(CHUNK * cc, CHUNK * (cc + 1))
                    sq = sq_next

                    # previous chunk's output copy + (at superchunk ends) its
                    # output DMA — pipelined here so the ACT queue never
                    # blocks on a not-yet-finished psO
                    if pending_out is not None:
                        p_psO, p_outB, p_csl, p_flush = pending_out
                        nc.scalar.copy(p_outB[:, p_csl], p_psO[:, :])
                        if p_flush is not None:
                            p_b0, p_nb = p_flush
                            nc.sync.dma_start(
                                out_d[p_b0:p_b0 + p_nb].rearrange(
                                    "b f c -> f b c"),
                                p_outB[:, :])

                    zt = []
                    for t_i, (c0, c1) in enumerate(MCHUNKS):
                        m = c1 - c0
                        psS = ps1.tile([m, CHUNK], f32, tag="s1")
                        nc.tensor.matmul(psS[:, :], m1a[:, c0:c1], sq[:, :],
                                         start=True, stop=False)
                        nc.tensor.matmul(psS[:, :], m1bc[:, c0:c1],
                                         xin[:, csl], start=False, stop=True)
                        z = zp.tile([m, CHUNK], f32r, tag="z")
                        nc.vector.scalar_tensor_tensor(
                            z[:, :], psS[:, :], 1.0, xrt[t_i][:, csl],
                            MUL, MUL)
                        zt.append(z)
                        if t_i == 0:
                            # head of the next chunk, early enough that its
                            # ACT squares land before that chunk's S1
                            if cc + 1 < SUP:
                                sq_next = issue_head(
                                    xin, xbf, slice(CHUNK * (cc + 1),
                                                    CHUNK * (cc + 2)))
                            elif si + 1 < len(scs):
                                sq_next = issue_head(pending[0], pending[1],
                                                     slice(0, CHUNK))

                    # previous chunk's ZW + psO, deferred so ZW never
                    # blocks the DVE queue head waiting on this chunk's P41
                    if pending_zw is not None:
                        p_psP, p_sl, p_outB, p_csl, p_flush = pending_zw
                        zw = zwp.tile([NP41, CHUNK], f32r, tag="zw")
                        nc.vector.scalar_tensor_tensor(
                            zw[:, :], p_psP[:, :], 1.0, wt41[:, p_sl],
                            MUL, MUL)
                        psO = po.tile([4, CHUNK], f32, tag="psO")
                        nc.tensor.matmul(psO[:, :], g2[:, :], zw[:, :],
                                         start=True, stop=True)
                        pending_out = (psO, p_outB, p_csl, p_flush)

                    psP = pp41.tile([NP41, CHUNK], f32, tag="p41")
                    for j, t_i in enumerate(range(4)):
                        nc.tensor.matmul(psP[:, :], g1t[t_i][:, :],
                                         zt[t_i][:, :],
                                         start=(j == 0), stop=(j == 3))
                    flush = (sc * (SCH // C), SCH // C) if cc == SUP - 1 else None
                    pending_zw = (psP, sl, outB, csl, flush)

            if pending_out is not None:
                p_psO, p_outB, p_csl, p_flush = pending_out
                nc.scalar.copy(p_outB[:, p_csl], p_psO[:, :])
                if p_flush is not None:
                    p_b0, p_nb = p_flush
                    nc.sync.dma_start(
                        out_d[p_b0:p_b0 + p_nb].rearrange("b f c -> f b c"),
                        p_outB[:, :])
            p_psP, p_sl, p_outB, p_csl, p_flush = pending_zw
            zw = zwp.tile([NP41, CHUNK], f32r, tag="zw")
            nc.vector.scalar_tensor_tensor(zw[:, :], p_psP[:, :], 1.0,
                                           wt41[:, p_sl], MUL, MUL)
            psO = po.tile([4, CHUNK], f32, tag="psO")
            nc.tensor.matmul(psO[:, :], g2[:, :], zw[:, :],
                             start=True, stop=True)
            nc.scalar.copy(p_outB[:, p_csl], psO[:, :])
            p_b0, p_nb = p_flush
            nc.sync.dma_start(
                out_d[p_b0:p_b0 + p_nb].rearrange("b f c -> f b c"),
                p_outB[:, :])
    nc.compile()
    return nc


_NC_CACHE = {}


def _get_nc(bpc=BPC, repeat=1):
    key = (bpc, repeat)
    if key not in _NC_CACHE:
        _NC_CACHE[key] = build_nc(bpc, repeat)
    return _NC_CACHE[key]


def make_in_maps(inputs, bpc=BPC, ncores=NCORES):
    import ml_dtypes
    a_i = np.ascontiguousarray(inputs["a_i"], dtype=np.float32)
    y = np.ascontiguousarray(inputs["node_attrs"], dtype=np.float32)
    consts = _build_consts(
        np.asarray(inputs["U3_l0"], np.float32), np.asarray(inputs["U2_l0"], np.float32),
        np.asarray(inputs["U1_l0"], np.float32), np.asarray(inputs["U3_l1"], np.float32),
        np.asarray(inputs["U2_l1"], np.float32), np.asarray(inputs["U1_l1"], np.float32))
    Wmap = np.concatenate([
        np.asarray(inputs["W3_l0"], np.float32), np.asarray(inputs["W2_l0"], np.float32),
        np.asarray(inputs["W1_l0"], np.float32), np.asarray(inputs["W3_l1"], np.float32),
        np.asarray(inputs["W2_l1"], np.float32), np.asarray(inputs["W1_l1"], np.float32)],
        axis=1)                                    # [E, 19, C]
    cf = np.zeros((NOFF, 2 * NCOLS_S1 + NOFF + 4 + 4 * NP41), np.float32)
    cf[:, 0:NCOLS_S1] = consts["M1a"]
    cf[0:NXIN, NCOLS_S1:2 * NCOLS_S1] = consts["M1bc"]
    cg0 = 2 * NCOLS_S1 + NOFF + 4
    cf[0:NP41, 2 * NCOLS_S1 + NOFF:cg0] = consts["G2"]
    for t in range(4):
        r0, r1 = MCHUNKS[t]
        cf[0:r1 - r0, cg0 + NP41 * t:cg0 + NP41 * (t + 1)] = consts["G1"][r0:r1]
    cf[0:17, 2 * NCOLS_S1:2 * NCOLS_S1 + NOFF] = consts["SelAB"]
    shared = {"CF": cf,
              "ZfillC": np.zeros((16, CHUNK * 4), np.float32)}
    in_maps = []
    for core in range(ncores):
        b0 = core * bpc
        asl = a_i[b0:b0 + bpc]
        xa = np.empty((17, bpc * C), np.float32)
        xa[:16] = asl.transpose(2, 0, 1).reshape(DIM_I, bpc * C)
        xa[16] = 1.0
        w = y[b0:b0 + bpc] @ Wmap.reshape(E, NW * C)   # [bpc, NW*C]
        w41 = w.reshape(bpc, NW, C)[:, KROW, :]        # [bpc, 41, C]
        m = dict(shared)
        m["xa"] = xa
        m["Wt41"] = np.ascontiguousarray(
            w41.transpose(1, 0, 2).reshape(NP41, bpc * C).astype(ml_dtypes.bfloat16))
        in_maps.append(m)
    return in_maps


def assemble_output(results, bpc=BPC):
    outs = []
    for r in results:
        o = r["out"]
        outs.append(np.concatenate(
            [o[:, 0, :], o[:, 1:4, :].transpose(0, 2, 1).reshape(bpc, 3 * C)],
            axis=1))
    return np.concatenate(outs, axis=0)


def kernel(**inputs):
    from concourse import bass_utils
    nc = _get_nc()
    in_maps = make_in_maps(inputs)
    res = bass_utils.run_bass_kernel_spmd(nc, in_maps, core_ids=list(range(NCORES)))
    return assemble_output(res.results)


# revision 8
# speedup vs baseline: 1.0678x; 1.0096x over previous
"""Trainium2 Bass kernel for the MACE-style SymmetricContraction MessageBlock.

Math (per sample s=(b,c), x = a_i[b,c,:] in R^16, w*[b,k,c] = node_attrs @ W*):
  out0[b,c]   = sum U3_l0[i1,i2,i3,k] x_i1 x_i2 x_i3 w3[k]
              + sum U2_l0[i1,i2,k]    x_i1 x_i2      w2[k]
              + sum U1_l0[i1,k]       x_i1           w1[k]
  out1[b,c,l] = same with the l1 U/W tensors.

Device pipeline per 512-sample chunk. Pair monomials x_a*x_b come from the
sum-of-squares identity x_a x_b = ((x_a+x_b)^2 - x_a^2 - x_b^2)/2; the x_a^2
corrections are folded into the coefficient rows on the host:
  sq   = square(SelAB.T @ x)                 # 120 rows (PE+ACT)
  xin  = [x^2 ; 0 ; x ; 1]                   # 49 rows; x^2 by Pool per chunk
  S1   = M1a'.T @ sq + M1bc'.T @ xin         # 431 cols, 4 m-chunks of <=119
  Z    = S1 * Xrep (f32r)                    # Xrep = x_{i1(col)} via
                                             #   broadcast SBUF->SBUF DMA
  P41  = G1.T @ Z                            # reduce over i1 -> 41 cols
  ZW   = P41 * wt41                          # wt41 pre-expanded at setup
  out4 = G2.T @ ZW                           # out0, out1 l=0..2

Engine split per chunk: PE 14 matmul passes; ACT 2 squares + out copy;
DVE 4 Z mults + ZW; DMA streams x and broadcasts Xrep (f32r).

Sharding: data-parallel over nodes, 128 nodes per core on 8 cores.
"""
import numpy as np

B, C, DIM_I, E = 1024, 128, 16, 10
NCORES = 8
BPC = B // NCORES
S_PER_CORE = BPC * C
CHUNK = 512

OFFDIAG = [(a, b) for a in range(DIM_I) for b in range(a + 1, DIM_I)]  # 120
NOFF = len(OFFDIAG)
NC_CUBIC = 26
NCOLS_C = DIM_I * NC_CUBIC          # 416
NCOLS_S1 = NCOLS_C + 11 + 4         # 431
NP41 = 41
NW = 19
# m-chunks of 4 i1-groups (4*26=104); last chunk also carries quad/lin cols
MCHUNKS = [(0, 104), (104, 208), (208, 312), (312, NCOLS_S1)]
NXIN = 49  # xin rows: 0..15 x^2, 16..31 zero, 32..47 x, 48 ones

# P41 col -> packed w row (w3_l0:0-4, w2_l0:5-6, w1_l0:7, w3_l1:8-14,
# w2_l1:15-17, w1_l1:18)
KROW = [0] * NP41
for _m in range(5):
    KROW[_m] = _m
for _l in range(3):
    for _k in range(7):
        KROW[5 + _l * 7 + _k] = 8 + _k
KROW[26], KROW[27] = 5, 6
for _l in range(3):
    for _k in range(3):
        KROW[28 + _l * 3 + _k] = 15 + _k
KROW[37] = 7
for _l in range(3):
    KROW[38 + _l] = 18


# ---------------------------------------------------------------- host consts
def _pair_coeff(U3_l0, U3_l1, U2_l0, U2_l1, a, b):
    """431-col coefficient row for the pair monomial x_a*x_b (a<=b)."""
    dup = a < b
    row = np.zeros(NCOLS_S1, np.float32)
    for i1 in range(DIM_I):
        c0 = i1 * NC_CUBIC
        row[c0:c0 + 5] = U3_l0[i1, a, b] + (U3_l0[i1, b, a] if dup else 0)
        for l in range(3):
            row[c0 + 5 + l * 7: c0 + 12 + l * 7] = (
                U3_l1[l, i1, a, b] + (U3_l1[l, i1, b, a] if dup else 0))
    row[416:418] = U2_l0[a, b] + (U2_l0[b, a] if dup else 0)
    for l in range(3):
        row[418 + l * 3: 421 + l * 3] = (
            U2_l1[l, a, b] + (U2_l1[l, b, a] if dup else 0))
    return row


def _build_consts(U3_l0, U2_l0, U1_l0, U3_l1, U2_l1, U1_l1):
    M1a_raw = np.zeros((NOFF, NCOLS_S1), np.float32)
    for p, (a, b) in enumerate(OFFDIAG):
        M1a_raw[p] = _pair_coeff(U3_l0, U3_l1, U2_l0, U2_l1, a, b)
    # sum-of-squares substitution: sq rows carry M1a/2; diagonal rows get
    # the -1/2 sum of all off-diag rows touching that index
    M1a = 0.5 * M1a_raw
    M1bc = np.zeros((NXIN, NCOLS_S1), np.float32)
    for i in range(DIM_I):
        corr = np.zeros(NCOLS_S1, np.float32)
        for p, (a, b) in enumerate(OFFDIAG):
            if a == i or b == i:
                corr += M1a_raw[p]
        # row i: x_i^2 coefficient
        M1bc[i] = _pair_coeff(U3_l0, U3_l1, U2_l0, U2_l1, i, i) - 0.5 * corr
    for i in range(DIM_I):
        # rows 32..47: linear x_i coefficients
        M1bc[32 + i, 427] = U1_l0[i, 0]
        for l in range(3):
            M1bc[32 + i, 428 + l] = U1_l1[l, i, 0]

    G1 = np.zeros((NCOLS_S1, NP41), np.float32)
    for i1 in range(DIM_I):
        for m in range(NC_CUBIC):
            G1[i1 * NC_CUBIC + m, m] = 1
    for j in range(11):
        G1[416 + j, 26 + j] = 1
    for j in range(4):
        G1[427 + j, 37 + j] = 1

    G2 = np.zeros((NP41, 4), np.float32)
    G2[0:5, 0] = 1
    G2[26:28, 0] = 1
    G2[37, 0] = 1
    for l in range(3):
        G2[5 + l * 7: 12 + l * 7, 1 + l] = 1
        G2[28 + l * 3: 31 + l * 3, 1 + l] = 1
        G2[38 + l, 1 + l] = 1

    SelAB = np.zeros((17, NOFF), np.float32)
    for p, (a, b) in enumerate(OFFDIAG):
        SelAB[a, p] += 1
        SelAB[b, p] += 1
    return dict(M1a=M1a, M1bc=M1bc, G1=G1, G2=G2, SelAB=SelAB)


# ---------------------------------------------------------------- bass program
def build_nc(bpc=BPC, repeat=1):
    import concourse.bass as bass
    import concourse.bacc as bacc
    import concourse.mybir as mybir
    import concourse.tile as tile

    s_core = bpc * C
    nchunk = s_core // CHUNK
    f32 = mybir.dt.float32
    f32r = mybir.dt.float32r
    bf16 = mybir.dt.bfloat16
    MUL = mybir.AluOpType.mult

    nc = bacc.Bacc("TRN2", target_bir_lowering=False, debug=False)

    xa_d = nc.dram_tensor("xa", [17, s_core], f32r, kind="ExternalInput")
    wt41_d = nc.dram_tensor("Wt41", [NP41, s_core], bf16, kind="ExternalInput")
    # packed f32r consts: m1a | m1bc | sab | g2 along the free dim
    cf_d = nc.dram_tensor("CF", [NOFF, 2 * NCOLS_S1 + NOFF + 4 + 4 * NP41],
                          f32r, kind="ExternalInput")
    zf_d = nc.dram_tensor("ZfillC", [16, CHUNK * 4], f32r, kind="ExternalInput")
    out_d = nc.dram_tensor("out", [bpc, 4, C], f32, kind="ExternalOutput")

    SUP = 4
    SCH = CHUNK * SUP

    with tile.TileContext(nc) as tc:
        with (
            tc.tile_pool(name="const", bufs=1) as cp,
            tc.tile_pool(name="xbfp", bufs=3) as xbfp,
            tc.tile_pool(name="sqp", bufs=4) as sqp,
            tc.tile_pool(name="s1cp", bufs=3) as s1cp,
            tc.tile_pool(name="xrp", bufs=2) as xrp,
            tc.tile_pool(name="zp", bufs=8) as zp,
            tc.tile_pool(name="zwp", bufs=3) as zwp,
            tc.tile_pool(name="obp", bufs=3) as obp,
            tc.tile_pool(name="pab", bufs=1, space="PSUM") as pab,
            tc.tile_pool(name="ps1", bufs=3, space="PSUM") as ps1,
            tc.tile_pool(name="pp41", bufs=3, space="PSUM") as pp41,
            tc.tile_pool(name="po", bufs=1, space="PSUM") as po,
        ):
            def ctile(name, shape, dram, dt=f32r):
                t = cp.tile(shape, dt, tag=name)
                nc.sync.dma_start(t[:, :], dram[:])
                return t

            cf = ctile("cf", [NOFF, 2 * NCOLS_S1 + NOFF + 4 + 4 * NP41], cf_d)
            m1a = cf[:, 0:NCOLS_S1]
            m1bc = cf[0:NXIN, NCOLS_S1:2 * NCOLS_S1]

            g2 = cf[0:NP41, 2 * NCOLS_S1 + NOFF:2 * NCOLS_S1 + NOFF + 4]
            sab = cf[0:17, 2 * NCOLS_S1:2 * NCOLS_S1 + NOFF]
            CG0 = 2 * NCOLS_S1 + NOFF + 4
            g1t = [cf[0:MCHUNKS[t][1] - MCHUNKS[t][0],
                      CG0 + NP41 * t:CG0 + NP41 * (t + 1)] for t in range(4)]

            # per-node path weights, precomputed on the host. DMA issued
            # after the first superchunk's input streams (ZW needs it last).
            wt41 = cp.tile([NP41, s_core], bf16, tag="wt41")

            # xin ring: [x^2 ; 0 ; x ; 1] per superchunk. Manual ring so the
            # zero band (read by M1bc rows 16:32, which are all-zero coeffs)
            # is written once; engine writes stay quadrant-aligned.
            xin_ring = []
            for i in range(3):
                xt = cp.tile([NXIN, SCH], f32r, tag=f"xin{i}")
                xin_ring.append(xt)

            def issue_inputs(sc):
                """Stream xin/xbf + Xrep broadcasts for superchunk sc."""
                ssl = slice(SCH * sc, SCH * (sc + 1))
                xin = xin_ring[issue_inputs.n % 3]
                issue_inputs.n += 1
                nc.sync.dma_start(xin[32:49, :], xa_d[:, ssl])
                xbf = xbfp.tile([17, SCH], f32r, tag="xbf")
                nc.scalar.dma_start(xbf[:, :], xa_d[:, ssl])
                xr = xrp.tile([119, 4 * SCH], f32r, tag="xr")
                first = issue_inputs.n == 1
                for t_i in range(4):
                    base = xbf[4 * t_i:4 * t_i + 4, :]
                    eng = nc.sync if t_i % 2 == 0 else nc.scalar
                    if first and t_i == 0:
                        # startup: chunk-sized pieces so chunk 0's Z starts
                        # after ~1/16 of the broadcast instead of all of it
                        for q in range(4):
                            b2 = base[:, CHUNK * q:CHUNK * (q + 1)]
                            srcap = bass.AP(
                                tensor=b2.tensor, offset=b2.offset,
                                ap=[list(b2.ap[0]), [0, NC_CUBIC],
                                    list(b2.ap[-1])])
                            eng.dma_start(
                                xr[0:104, SCH * t_i + CHUNK * q:
                                   SCH * t_i + CHUNK * (q + 1)], srcap)
                        continue
                    srcap = bass.AP(tensor=base.tensor, offset=base.offset,
                                    ap=[list(base.ap[0]), [0, NC_CUBIC],
                                        list(base.ap[-1])])
                    eng.dma_start(xr[0:104, SCH * t_i:SCH * (t_i + 1)], srcap)
                ones = xbf[16:17, :]
                onesap = bass.AP(tensor=ones.tensor, offset=ones.offset,
                                 ap=[list(ones.ap[0]), [0, 15],
                                     list(ones.ap[-1])])
                nc.scalar.dma_start(xr[104:119, 3 * SCH:4 * SCH], onesap)
                xrt = [xr[0:MCHUNKS[t][1] - MCHUNKS[t][0],
                          SCH * t:SCH * (t + 1)] for t in range(4)]
                return xin, xbf, xrt

            def issue_head(xin_t, xbf_t, csl_t):
                """Chunk head: pair sums (PE), x^2 rows and sq (ACT). x^2 is
                squared from the bf16 x copy: it lives on partitions 0:16,
                matching the destination (engines cannot shift partitions).
                Issued one chunk ahead of the S1 stage that uses it."""
                psAB = pab.tile([NOFF, CHUNK], f32, tag="psAB")
                nc.tensor.matmul(psAB[:, :], sab[:, :], xbf_t[0:17, csl_t],
                                 start=True, stop=True)
                nc.gpsimd.tensor_tensor(xin_t[0:16, csl_t],
                                        xbf_t[0:16, csl_t],
                                        xbf_t[0:16, csl_t], MUL)
                sq = sqp.tile([NOFF, CHUNK], f32r, tag="sq")
                nc.scalar.square(sq[:, :], psAB[:, :])
                return sq

            # ---- main loop over superchunks of SUP chunks. Input streams
            # are issued one superchunk ahead (so they never queue behind
            # output DMAs) and chunk heads one chunk ahead (so sq/x^2 are
            # ready when S1 runs).
            scs = [s for _ in range(repeat) for s in range(nchunk // SUP)]
            issue_inputs.n = 0
            pending = issue_inputs(scs[0])
            nc.sync.dma_start(wt41[:, :], wt41_d[:])
            for xt in xin_ring:
                nc.scalar.dma_start(xt[16:32, :], zf_d[:])
            sq_next = issue_head(pending[0], pending[1], slice(0, CHUNK))
            pending_out = None
            pending_zw = None
            for si, sc in enumerate(scs):
                xin, xbf, xrt = pending
                if si + 1 < len(scs):
                    pending = issue_inputs(scs[si + 1])
                outB = obp.tile([4, SCH], f32, tag="outB")

                for cc in range(SUP):
                    ch = SUP * sc + cc
                    sl = slice(CHUNK * ch, CHUNK * (ch + 1))
                    csl = slice(CHUNK * cc, CHUNK * (cc + 1))
                    sq = sq_next

                    zt = []
                    for t_i, (c0, c1) in enumerate(MCHUNKS):
                        m = c1 - c0
                        psS = ps1.tile([m, CHUNK], f32, tag="s1")
                        nc.tensor.matmul(psS[:, :], m1a[:, c0:c1], sq[:, :],
                                         start=True, stop=False)
                        nc.tensor.matmul(psS[:, :], m1bc[:, c0:c1],
                                         xin[:, csl], start=False, stop=True)
                        z = zp.tile([m, CHUNK], f32r, tag="z")
                        nc.vector.scalar_tensor_tensor(
                            z[:, :], psS[:, :], 1.0, xrt[t_i][:, csl],
                            MUL, MUL)
                        zt.append(z)
                        if t_i == 0:
                            # head of the next chunk, early enough that its
                            # ACT squares land before that chunk's S1
                            if cc + 1 < SUP:
                                sq_next = issue_head(
                                    xin, xbf, slice(CHUNK * (cc + 1),
                                                    CHUNK * (cc + 2)))
                            elif si + 1 < len(scs):
                                sq_next = issue_head(pending[0], pending[1],
                                                     slice(0, CHUNK))

                    # output copy two chunks back, after this chunk's sq
                    # head so the ACT queue serves sq first
                    if pending_out is not None:
                        p_psO, p_outB, p_csl, p_flush = pending_out
                        nc.scalar.copy(p_outB[:, p_csl], p_psO[:, :])
                        if p_flush is not None:
                            p_b0, p_nb = p_flush
                            nc.sync.dma_start(
                                out_d[p_b0:p_b0 + p_nb].rearrange(
                                    "b f c -> f b c"),
                                p_outB[:, :])

                    psP = pp41.tile([NP41, CHUNK], f32, tag="p41")
                    for j, t_i in enumerate(range(4)):
                        nc.tensor.matmul(psP[:, :], g1t[t_i][:, :],
                                         zt[t_i][:, :],
                                         start=(j == 0), stop=(j == 3))

                    # previous chunk's ZW + psO, deferred so ZW never
                    # blocks the DVE queue head waiting on this chunk's P41
                    if pending_zw is not None:
                        p_psP, p_sl, p_outB, p_csl, p_flush = pending_zw
                        zw = zwp.tile([NP41, CHUNK], f32r, tag="zw")
                        nc.vector.scalar_tensor_tensor(
                            zw[:, :], p_psP[:, :], 1.0, wt41[:, p_sl],
                            MUL, MUL)
                        psO = po.tile([4, CHUNK], f32, tag="psO")
                        nc.tensor.matmul(psO[:, :], g2[:, :], zw[:, :],
                                         start=True, stop=True)
                        pending_out = (psO, p_outB, p_csl, p_flush)
                    flush = (sc * (SCH // C), SCH // C) if cc == SUP - 1 else None
                    pending_zw = (psP, sl, outB, csl, flush)

            if pending_out is not None:
                p_psO, p_outB, p_csl, p_flush = pending_out
                nc.scalar.copy(p_outB[:, p_csl], p_psO[:, :])
                if p_flush is not None:
                    p_b0, p_nb = p_flush
                    nc.sync.dma_start(
                        out_d[p_b0:p_b0 + p_nb].rearrange("b f c -> f b c"),
                        p_outB[:, :])
            p_psP, p_sl, p_outB, p_csl, p_flush = pending_zw
            zw = zwp.tile([NP41, CHUNK], f32r, tag="zw")
            nc.vector.scalar_tensor_tensor(zw[:, :], p_psP[:, :], 1.0,
                                           wt41[:, p_sl], MUL, MUL)
            psO = po.tile([4, CHUNK], f32, tag="psO")
            nc.tensor.matmul(psO[:, :], g2[:, :], zw[:, :],
                             start=True, stop=True)
            nc.scalar.copy(p_outB[:, p_csl], psO[:, :])
            p_b0, p_nb = p_flush
            nc.sync.dma_start(
                out_d[p_b0:p_b0 + p_nb].rearrange("b f c -> f b c"),
                p_outB[:, :])
    nc.compile()
    return nc


_NC_CACHE = {}


def _get_nc(bpc=BPC, repeat=1):
    key = (bpc, repeat)
    if key not in _NC_CACHE:
        _NC_CACHE[key] = build_nc(bpc, repeat)
    return _NC_CACHE[key]


def make_in_maps(inputs, bpc=BPC, ncores=NCORES):
    import ml_dtypes
    a_i = np.ascontiguousarray(inputs["a_i"], dtype=np.float32)
    y = np.ascontiguousarray(inputs["node_attrs"], dtype=np.float32)
    consts = _build_consts(
        np.asarray(inputs["U3_l0"], np.float32), np.asarray(inputs["U2_l0"], np.float32),
        np.asarray(inputs["U1_l0"], np.float32), np.asarray(inputs["U3_l1"], np.float32),
        np.asarray(inputs["U2_l1"], np.float32), np.asarray(inputs["U1_l1"], np.float32))
    Wmap = np.concatenate([
        np.asarray(inputs["W3_l0"], np.float32), np.asarray(inputs["W2_l0"], np.float32),
        np.asarray(inputs["W1_l0"], np.float32), np.asarray(inputs["W3_l1"], np.float32),
        np.asarray(inputs["W2_l1"], np.float32), np.asarray(inputs["W1_l1"], np.float32)],
        axis=1)                                    # [E, 19, C]
    cf = np.zeros((NOFF, 2 * NCOLS_S1 + NOFF + 4 + 4 * NP41), np.float32)
    cf[:, 0:NCOLS_S1] = consts["M1a"]
    cf[0:NXIN, NCOLS_S1:2 * NCOLS_S1] = consts["M1bc"]
    cg0 = 2 * NCOLS_S1 + NOFF + 4
    cf[0:NP41, 2 * NCOLS_S1 + NOFF:cg0] = consts["G2"]
    for t in range(4):
        r0, r1 = MCHUNKS[t]
        cf[0:r1 - r0, cg0 + NP41 * t:cg0 + NP41 * (t + 1)] = consts["G1"][r0:r1]
    cf[0:17, 2 * NCOLS_S1:2 * NCOLS_S1 + NOFF] = consts["SelAB"]
    shared = {"CF": cf,
              "ZfillC": np.zeros((16, CHUNK * 4), np.float32)}
    in_maps = []
    for core in range(ncores):
        b0 = core * bpc
        asl = a_i[b0:b0 + bpc]
        xa = np.empty((17, bpc * C), np.float32)
        xa[:16] = asl.transpose(2, 0, 1).reshape(DIM_I, bpc * C)
        xa[16] = 1.0
        w = y[b0:b0 + bpc] @ Wmap.reshape(E, NW * C)   # [bpc, NW*C]
        w41 = w.reshape(bpc, NW, C)[:, KROW, :]        # [bpc, 41, C]
        m = dict(shared)
        m["xa"] = xa
        m["Wt41"] = np.ascontiguousarray(
            w41.transpose(1, 0, 2).reshape(NP41, bpc * C).astype(ml_dtypes.bfloat16))
        in_maps.append(m)
    return in_maps


def assemble_output(results, bpc=BPC):
    outs = []
    for r in results:
        o = r["out"]
        outs.append(np.concatenate(
            [o[:, 0, :], o[:, 1:4, :].transpose(0, 2, 1).reshape(bpc, 3 * C)],
            axis=1))
    return np.concatenate(outs, axis=0)


def kernel(**inputs):
    from concourse import bass_utils
    nc = _get_nc()
    in_maps = make_in_maps(inputs)
    res = bass_utils.run_bass_kernel_spmd(nc, in_maps, core_ids=list(range(NCORES)))
    return assemble_output(res.results)
